# revision 45
# baseline (speedup 1.0000x reference)
"""Trainium2 Bass kernel for nn_FEM_33251636806316 (sparse_attention).

Data-parallel over batch: 64 items -> 8 NeuronCores x 8 items each.
All heavy matmuls in fp16 (full PE rate), psum fp32, residual stream and
LN statistics in float32r (fp32 storage, full-rate matmul for N>=256).
"""

import math

import numpy as np

# ---------------------------------------------------------------- constants
B = 64
HW = 14
EMB = 512
CUR = 1024
CH = 256  # conv channels
M = 256  # FAVOR+ features
T = 4 * HW * HW  # 784 tokens
N_CORES = 8
ITEMS = B // N_CORES  # 8 per core
NEG_LN16 = -math.log(math.sqrt(M))  # -ln(16)

# conv layer chunk table: (chunk_base, n_in_chunks) in the packed conv weight
_CONV_CHUNKS = [(0, 4), (4, 4), (8, 4), (12, 4), (16, 2)]

# debug: replace Gelu by Identity (CoreSim has no Gelu model)
GELU_IDENTITY = False


# ---------------------------------------------------------------- host prep
def _prep_arrays(a, b, c, params, w):
    """Host-side marshalling: weight folding, layout transforms, fp16 casts."""
    p = params
    f16 = np.float16
    f32 = np.float32

    def to16(x):
        return np.ascontiguousarray(np.asarray(x), dtype=f16)

    def to32(x):
        return np.ascontiguousarray(np.asarray(x), dtype=f32)

    arrs = {}
    # inputs
    arrs["a_t"] = to16(np.asarray(a).transpose(0, 2, 1))  # [B, 512, 784]
    arrs["b_in"] = to32(b)  # [B, 196, 1024]
    arrs["c_in"] = to32(np.asarray(c).reshape(B, CH, 28 * 28))  # [B, 256, 784]

    # conv weights: fold BN scale into W, transpose to [ic, tap, oc], chunked
    chunks = []
    bias = np.zeros((5, CH), f32)
    for i in range(1, 6):
        wi = np.asarray(p[f"cbr{i}_w"], f32)  # [oc, ic, 3, 3]
        si = np.asarray(p[f"cbr{i}_s"], f32)
        bi = np.asarray(p[f"cbr{i}_b"], f32)
        wi = wi * si[:, None, None, None]
        bias[i - 1] = bi
        wt = wi.transpose(1, 2, 3, 0).reshape(wi.shape[1], 9, CH)  # [ic,9,oc]
        nch = wi.shape[1] // 128
        chunks.append(wt.reshape(nch, 128, 9, CH))
    arrs["convw"] = to16(np.concatenate(chunks, 0))  # [18, 128, 9, 256]
    arrs["convb"] = bias  # [5, 256]

    arrs["cf1w"] = to16(p["cf_w1"])  # [768, 512]
    arrs["cf1b"] = to32(p["cf_b1"])
    arrs["cf2w"] = to16(p["cf_w2"])
    arrs["cf2b"] = to32(p["cf_b2"])

    # LN1 folded into kqv
    g1 = np.asarray(p["ln1_g"], f32)
    b1 = np.asarray(p["ln1_b"], f32)
    kqvw = np.asarray(p["kqv_w"], f32)
    kqvb = np.asarray(p["kqv_b"], f32)
    arrs["kqvw"] = to16(kqvw * g1[:, None])  # [512, 1536]
    c2 = b1 @ kqvw + kqvb  # [1536]
    arrs["kqvc2"] = to32(c2[:1024])  # k,q biases
    arrs["c2vbc"] = to16(np.tile(c2[1024:1536][None, :], (128, 1)))  # [128,512]

    arrs["wT"] = to16(np.asarray(w, f32).T)  # [512, 256]
    arrs["projw"] = to16(p["proj_w"])
    arrs["projb"] = to32(p["proj_b"])

    # LN2 folded into mlp1
    g2 = np.asarray(p["ln2_g"], f32)
    b2 = np.asarray(p["ln2_b"], f32)
    m1w = np.asarray(p["mlp_w1"], f32)
    arrs["mlp1w"] = to16(m1w * g2[:, None])
    arrs["mlp1c2"] = to32(b2 @ m1w + np.asarray(p["mlp_b1"], f32))
    arrs["mlp2w"] = to16(p["mlp_w2"])
    arrs["b2bc"] = to32(np.tile(np.asarray(p["mlp_b2"], f32)[None, :], (128, 1)))

    arrs["i512"] = np.eye(512, dtype=f32)

    arrs["caw1"] = to16(p["ca_w1"])  # [256, 16]
    arrs["caw2"] = to16(p["ca_w2"])  # [16, 256]

    # spatial-attention banded matrices: band[ch*7+kx][y_in, y_out]
    saw = np.asarray(p["sa_w"], f32)[0]  # [2, 7, 7]
    bands = np.zeros((14, 28, 28), f32)
    for ch in range(2):
        for kx in range(7):
            for yo in range(28):
                for yi in range(max(0, yo - 3), min(28, yo + 4)):
                    bands[ch * 7 + kx, yi, yo] = saw[ch, yi - yo + 3, kx]
    bands[0:7] *= 1.0 / CH  # fold the channel-mean 1/256 into ch-0 bands
    arrs["bands"] = to16(bands)
    return arrs


# ---------------------------------------------------------------- bass build
def _build_bass(n_items):
    from contextlib import ExitStack

    import concourse.bass as bass
    import concourse.tile as tile
    from concourse import bacc, mybir
    from concourse.masks import make_identity

    f16 = mybir.dt.float16
    f32 = mybir.dt.float32
    f32r = mybir.dt.float32r
    AX = mybir.AxisListType
    ALU = mybir.AluOpType
    ACTF = mybir.ActivationFunctionType

    nc = bacc.Bacc("TRN2", target_bir_lowering=False, debug=False)

    # ---- dram tensors
    d_a = nc.dram_tensor("a_t", [n_items, EMB, T], f16, kind="ExternalInput").ap()
    d_b = nc.dram_tensor("b_in", [n_items, 196, CUR], f32, kind="ExternalInput").ap()
    d_c = nc.dram_tensor("c_in", [n_items, CH, 784], f32, kind="ExternalInput").ap()
    d_convw = nc.dram_tensor("convw", [18, 128, 9, CH], f16, kind="ExternalInput").ap()
    d_convb = nc.dram_tensor("convb", [5, CH], f32, kind="ExternalInput").ap()
    d_cf1w = nc.dram_tensor("cf1w", [768, 512], f16, kind="ExternalInput").ap()
    d_cf1b = nc.dram_tensor("cf1b", [512], f32, kind="ExternalInput").ap()
    d_cf2w = nc.dram_tensor("cf2w", [512, 512], f16, kind="ExternalInput").ap()
    d_cf2b = nc.dram_tensor("cf2b", [512], f32, kind="ExternalInput").ap()
    d_kqvw = nc.dram_tensor("kqvw", [512, 1536], f16, kind="ExternalInput").ap()
    d_kqvc2 = nc.dram_tensor("kqvc2", [1024], f32, kind="ExternalInput").ap()
    d_c2vbc = nc.dram_tensor("c2vbc", [128, 512], f16, kind="ExternalInput").ap()
    d_wT = nc.dram_tensor("wT", [512, 256], f16, kind="ExternalInput").ap()
    d_projw = nc.dram_tensor("projw", [512, 512], f16, kind="ExternalInput").ap()
    d_projb = nc.dram_tensor("projb", [512], f32, kind="ExternalInput").ap()
    d_mlp1w = nc.dram_tensor("mlp1w", [512, 512], f16, kind="ExternalInput").ap()
    d_mlp1c2 = nc.dram_tensor("mlp1c2", [512], f32, kind="ExternalInput").ap()
    d_mlp2w = nc.dram_tensor("mlp2w", [512, 512], f16, kind="ExternalInput").ap()
    d_b2bc = nc.dram_tensor("b2bc", [128, 512], f32, kind="ExternalInput").ap()
    d_i512 = nc.dram_tensor("i512", [512, 512], f32r, kind="ExternalInput").ap()
    d_caw1 = nc.dram_tensor("caw1", [CH, 16], f16, kind="ExternalInput").ap()
    d_caw2 = nc.dram_tensor("caw2", [16, CH], f16, kind="ExternalInput").ap()
    d_bands = nc.dram_tensor("bands", [14, 28, 28], f16, kind="ExternalInput").ap()
    d_out = nc.dram_tensor("out", [n_items, T, EMB], f32, kind="ExternalOutput").ap()

    with tile.TileContext(nc) as tc:
        with ExitStack() as ctx:
            with nc.allow_low_precision(reason="intentional fp16 matmul pipeline"):
                _emit(tc, ctx, nc, n_items, locals())
    nc.compile()
    return nc


def _emit(tc, ctx, nc, n_items, d):
    import concourse.bass as bass
    import concourse.bass_isa as bass_isa
    from concourse import mybir
    from concourse.masks import make_identity

    f16 = mybir.dt.float16
    f32 = mybir.dt.float32
    f32r = mybir.dt.float32r
    AX = mybir.AxisListType
    ALU = mybir.AluOpType
    ACTF = mybir.ActivationFunctionType
    GELU = ACTF.Identity if GELU_IDENTITY else ACTF.Gelu

    pool = lambda name, bufs: ctx.enter_context(tc.tile_pool(name=name, bufs=bufs))
    wp = pool("weights", 1)  # resident weights
    cwp = pool("convw", 2)  # streamed conv weights
    cvp = pool("convact", 1)  # conv activations
    stg = pool("stage", 1)  # input staging
    tkp = pool("tok", 1)  # token activations
    sqp = pool("sq", 2)  # small square/tmp tiles
    rwp = pool("rows", 1)  # [1, x] rows
    smp = pool("small", 1)  # small per-item tensors
    otp = pool("outsb", 2)  # output staging
    psp = ctx.enter_context(tc.tile_pool(name="psum", bufs=3, space="PSUM"))
    psr = ctx.enter_context(tc.tile_pool(name="psrow", bufs=2, space="PSUM"))

    dma = nc.sync.dma_start

    # ---------------- resident weights
    w_cf1 = wp.tile([128, 6, 512], f16, tag="w_cf1")
    dma(w_cf1, d["d_cf1w"].rearrange("(c p) n -> p c n", p=128))
    w_cf2 = wp.tile([128, 4, 512], f16, tag="w_cf2")
    dma(w_cf2, d["d_cf2w"].rearrange("(c p) n -> p c n", p=128))
    w_kqv = wp.tile([128, 4, 1536], f16, tag="w_kqv")
    dma(w_kqv, d["d_kqvw"].rearrange("(c p) n -> p c n", p=128))
    w_wT = wp.tile([128, 4, 256], f16, tag="w_wT")
    dma(w_wT, d["d_wT"].rearrange("(c p) n -> p c n", p=128))
    w_proj = wp.tile([128, 4, 512], f16, tag="w_proj")
    dma(w_proj, d["d_projw"].rearrange("(c p) n -> p c n", p=128))
    w_mlp1 = wp.tile([128, 4, 512], f16, tag="w_mlp1")
    dma(w_mlp1, d["d_mlp1w"].rearrange("(c p) n -> p c n", p=128))
    w_mlp2 = wp.tile([128, 4, 512], f16, tag="w_mlp2")
    dma(w_mlp2, d["d_mlp2w"].rearrange("(c p) n -> p c n", p=128))
    w_i512 = wp.tile([128, 4, 512], f32r, tag="w_i512")
    dma(w_i512, d["d_i512"].rearrange("(c p) n -> p c n", p=128))
    w_c2vbc = wp.tile([128, 512], f16, tag="w_c2vbc")
    dma(w_c2vbc, d["d_c2vbc"])
    w_b2bc = wp.tile([128, 512], f32, tag="w_b2bc")
    dma(w_b2bc, d["d_b2bc"])
    w_caw1 = wp.tile([128, 2, 16], f16, tag="w_caw1")
    dma(w_caw1, d["d_caw1"].rearrange("(c p) n -> p c n", p=128))
    w_caw2 = wp.tile([16, 256], f16, tag="w_caw2")
    dma(w_caw2, d["d_caw2"])
    w_bands = wp.tile([28, 14, 28], f16, tag="w_bands")
    dma(w_bands, d["d_bands"].rearrange("i p j -> p i j"))
    b_conv = wp.tile([128, 5, 2], f32, tag="b_conv")
    dma(b_conv, d["d_convb"].rearrange("l (m p) -> p l m", p=128))
    b_cf1 = wp.tile([128, 4], f32, tag="b_cf1")
    dma(b_cf1, d["d_cf1b"].rearrange("(m p) -> p m", p=128))
    b_cf2 = wp.tile([128, 4], f32, tag="b_cf2")
    dma(b_cf2, d["d_cf2b"].rearrange("(m p) -> p m", p=128))
    b_kqv = wp.tile([128, 8], f32, tag="b_kqv")
    dma(b_kqv, d["d_kqvc2"].rearrange("(m p) -> p m", p=128))
    b_proj = wp.tile([128, 4], f32, tag="b_proj")
    dma(b_proj, d["d_projb"].rearrange("(m p) -> p m", p=128))
    b_mlp1 = wp.tile([128, 4], f32, tag="b_mlp1")
    dma(b_mlp1, d["d_mlp1c2"].rearrange("(m p) -> p m", p=128))

    ident = wp.tile([128, 128], f16, tag="ident")
    make_identity(nc, ident)
    ones16_col = wp.tile([128, 1], f16, tag="ones16_col")
    nc.gpsimd.memset(ones16_col, 1.0)
    neghalf_col = wp.tile([128, 1], f16, tag="neghalf_col")
    nc.gpsimd.memset(neghalf_col, -0.5)
    ones16_row = wp.tile([1, 256], f16, tag="ones16_row")
    nc.gpsimd.memset(ones16_row, 1.0)
    ones32_col_f = wp.tile([128, 1], f32, tag="ones32_col")
    nc.gpsimd.memset(ones32_col_f, 1.0)
    ones32_col = ones32_col_f.bitcast(f32r)
    ones32_row_f = wp.tile([1, 128], f32, tag="ones32_row")
    nc.gpsimd.memset(ones32_row_f, 1.0)
    ones32_row = ones32_row_f.bitcast(f32r)
    eps_row = wp.tile([1, 1], f32, tag="eps_row")
    nc.gpsimd.memset(eps_row, 1e-5)
    negln16_col = wp.tile([128, 1], f32, tag="negln16_col")
    nc.gpsimd.memset(negln16_col, NEG_LN16)

    HSL = [slice(0, 392), slice(392, 784)]  # token halves

    # =============================================================== items
    for it in range(n_items):
        # ---------------- input loads
        tb0 = stg.tile([128, 1024], f32, tag="bstage")
        dma(tb0, d["d_b"][it, 0:128, :])
        tc0 = stg.tile([128, 2, 784], f32, tag="c0")
        dma(tc0, d["d_c"][it].rearrange("(cj p) x -> p cj x", p=128))
        aT = stg.tile([128, 4, 784], f16, tag="aT")
        dma(aT, d["d_a"][it].rearrange("(cj p) t -> p cj t", p=128))

        # ---------------- pixel shuffle b -> t_pad
        b16_0 = stg.tile([128, 1024], f16, tag="b16_0")
        nc.scalar.activation(b16_0, tb0, ACTF.Copy)
        tb1 = stg.tile([128, 1024], f32, tag="bstage")
        dma(tb1[:68], d["d_b"][it, 128:196, :])
        b16_1 = stg.tile([68, 1024], f16, tag="b16_1")
        nc.scalar.activation(b16_1, tb1[:68], ACTF.Copy)
        t_pad = cvp.tile([128, 2, 30, 30], f16, tag="t_pad")
        nc.gpsimd.memset(t_pad, 0.0)
        bv0 = b16_0.rearrange("p (c g) -> p g c", g=4)
        bv1 = b16_1.rearrange("p (c g) -> p g c", g=4)
        for cj in range(2):
            for g in range(4):
                d1, d2 = g // 2, g % 2
                ps = psp.tile([128, 196], f16, tag="ps16")
                nc.tensor.transpose(
                    ps[:, 0:128], bv0[:, g, cj * 128:(cj + 1) * 128], ident)
                nc.tensor.transpose(
                    ps[:, 128:196], bv1[:, g, cj * 128:(cj + 1) * 128],
                    ident[:68, :68])
                dst = t_pad[:, cj, 1:29, 1:29].rearrange(
                    "p (i a) (j b) -> p a b i j", a=2, b=2)[:, d1, d2]
                nc.scalar.activation(dst, ps.rearrange("p (i j) -> p i j", j=14),
                                     ACTF.Copy)

        # ---------------- c load/pad + channel attention
        c_pad = cvp.tile([128, 2, 30, 30], f16, tag="c_pad")
        nc.gpsimd.memset(c_pad, 0.0)
        for cj in range(2):
            nc.scalar.activation(
                c_pad[:, cj, 1:29, 1:29],
                tc0[:, cj].rearrange("p (x y) -> p x y", y=28), ACTF.Copy)
        cred = smp.tile([128, 2, 2], f32, tag="cred")
        for cj in range(2):
            nc.vector.reduce_sum(cred[:, cj, 0:1], tc0[:, cj], axis=AX.X)
            nc.vector.reduce_max(cred[:, cj, 1:2], tc0[:, cj], axis=AX.X)
        v2 = smp.tile([128, 2, 2], f16, tag="v2")
        for cj in range(2):
            nc.vector.tensor_scalar_mul(v2[:, cj, 0:1], cred[:, cj, 0:1], 1.0 / 784)
            nc.vector.tensor_copy(v2[:, cj, 1:2], cred[:, cj, 1:2])
        psh = psp.tile([16, 2], f32, tag="ps")
        nc.tensor.matmul(psh, w_caw1[:, 0], v2[:, 0], start=True, stop=False)
        nc.tensor.matmul(psh, w_caw1[:, 1], v2[:, 1], start=False, stop=True)
        h16 = smp.tile([16, 2], f16, tag="h16")
        nc.scalar.activation(h16, psh, ACTF.Relu)
        cc = smp.tile([128, 2, 1], f32, tag="cc")
        for cj in range(2):
            ps2 = psp.tile([128, 1], f32, tag="ps")
            nc.tensor.matmul(ps2, w_caw2[:, cj * 128:(cj + 1) * 128], h16[:, 0:1],
                             start=True, stop=False)
            nc.tensor.matmul(ps2, w_caw2[:, cj * 128:(cj + 1) * 128], h16[:, 1:2],
                             start=False, stop=True)
            nc.scalar.activation(cc[:, cj], ps2, ACTF.Sigmoid)

        # ---------------- spatial attention
        tadd = smp.tile([128, 28, 28], f16, tag="tadd")
        nc.vector.tensor_add(
            tadd, t_pad[:, 0, 1:29, 1:29], t_pad[:, 1, 1:29, 1:29])
        tmax = smp.tile([128, 28, 28], f16, tag="tmax")
        nc.vector.tensor_tensor(
            tmax, t_pad[:, 0, 1:29, 1:29], t_pad[:, 1, 1:29, 1:29], ALU.max)
        maxall = smp.tile([128, 28, 28], f16, tag="maxall")
        nc.gpsimd.partition_all_reduce(maxall, tmax, 128, bass_isa.ReduceOp.max)
        maxrow = maxall[0:1]
        meanrow = rwp.tile([1, 2, 392], f16, tag="meanrow")
        for h in range(2):
            psm = psr.tile([1, 14, 28], f32, tag="psrow")
            nc.tensor.matmul(
                psm, ones16_col, tadd[:, h * 14:(h + 1) * 14, :],
                start=True, stop=True)
            nc.scalar.activation(meanrow[:, h], psm.rearrange("o i j -> o (i j)"),
                                 ACTF.Copy)
        sa_t = smp.tile([28, 2, 34], f16, tag="sa_t")
        nc.gpsimd.memset(sa_t, 0.0)
        dma(sa_t[:, 0, 3:31], meanrow.rearrange("o h x -> o (h x)"))
        dma(sa_t[:, 1, 3:31], maxrow.rearrange("o x y -> o (x y)"))
        pssa = psp.tile([28, 28], f32, tag="ps")
        for ch in range(2):
            for kx in range(7):
                nc.tensor.matmul(
                    pssa, w_bands[:, ch * 7 + kx, :], sa_t[:28, ch, kx:kx + 28],
                    start=(ch == 0 and kx == 0), stop=(ch == 1 and kx == 6))
        ts28 = smp.tile([28, 28], f16, tag="ts28")
        nc.scalar.activation(ts28, pssa, ACTF.Sigmoid)
        tsrow = rwp.tile([1, 784], f16, tag="tsrow")
        dma(tsrow, ts28.rearrange("p x -> p x"))

        # ---------------- t_x = t * cc, c_x = c * ts
        tx_pad = cvp.tile([128, 2, 30, 30], f16, tag="tx_pad")
        for cj in range(2):
            nc.vector.tensor_scalar_mul(tx_pad[:, cj], t_pad[:, cj], cc[:, cj])
        cx_pad = cvp.tile([128, 2, 30, 30], f16, tag="cx_pad")
        nc.gpsimd.memset(cx_pad, 0.0)
        for h in range(2):
            psts = psp.tile([128, 14, 28], f32, tag="ps")
            nc.tensor.matmul(
                psts, ones16_row[:, 0:128],
                tsrow[:, h * 392:(h + 1) * 392].rearrange("o (i j) -> o i j", j=28),
                start=True, stop=True)
            for cj in range(2):
                r0 = 1 + h * 14
                nc.vector.tensor_tensor(
                    cx_pad[:, cj, r0:r0 + 14, 1:29],
                    c_pad[:, cj, r0:r0 + 14, 1:29], psts, ALU.mult)

        # ---------------- conv blocks
        def conv_block(lidx, in_specs, out_write):
            cb, nch = _CONV_CHUNKS[lidx]
            wt = []
            for half in range((nch + 1) // 2):
                cw = cwp.tile([128, 2, 9, CH], f16, tag="cw")
                n = min(2, nch - half * 2)
                dma(cw[:, 0:n],
                    d["d_convw"][cb + half * 2: cb + half * 2 + n].rearrange(
                        "c p t o -> p c t o"))
                wt.append(cw)
            for m in range(2):
                for h in range(2):
                    ps = psp.tile([128, 14, 28], f32, tag="ps")
                    idx = 0
                    last = nch * 9 - 1
                    for kc in range(nch):
                        tile_in, cj = in_specs[kc]
                        for dy in range(3):
                            for dx in range(3):
                                lhsT = wt[kc // 2][:, kc % 2, dy * 3 + dx,
                                                   m * 128:(m + 1) * 128]
                                rhs = tile_in[:, cj, h * 14 + dy: h * 14 + dy + 14,
                                              dx:dx + 28]
                                nc.tensor.matmul(ps, lhsT, rhs,
                                                 start=(idx == 0), stop=(idx == last))
                                idx += 1
                    out_write(m, h, ps)

        def pad_writer(dst, lidx):
            def write(m, h, ps):
                o = dst[:, m, 1 + h * 14: 15 + h * 14, 1:29]
                if h == 0:
                    nc.scalar.activation(o, ps, ACTF.Relu, bias=b_conv[:, lidx, m:m+1])
                else:
                    nc.vector.tensor_scalar(o, ps, b_conv[:, lidx, m:m+1], 0.0,
                                            op0=ALU.add, op1=ALU.max)
            return write

        x1_pad = cvp.tile([128, 2, 30, 30], f16, tag="x1_pad")
        nc.gpsimd.memset(x1_pad, 0.0)
        conv_block(0, [(tx_pad, 0), (tx_pad, 1), (cx_pad, 0), (cx_pad, 1)],
                   pad_writer(x1_pad, 0))
        u_pad = cvp.tile([128, 2, 30, 30], f16, tag="tx_pad")
        nc.gpsimd.memset(u_pad, 0.0)
        conv_block(1, [(t_pad, 0), (t_pad, 1), (x1_pad, 0), (x1_pad, 1)],
                   pad_writer(u_pad, 1))
        cx2_pad = cvp.tile([128, 2, 30, 30], f16, tag="t_pad")
        nc.gpsimd.memset(cx2_pad, 0.0)
        conv_block(2, [(c_pad, 0), (c_pad, 1), (x1_pad, 0), (x1_pad, 1)],
                   pad_writer(cx2_pad, 2))
        x2_pad = cvp.tile([128, 2, 30, 30], f16, tag="cx_pad")
        nc.gpsimd.memset(x2_pad, 0.0)
        conv_block(3, [(u_pad, 0), (u_pad, 1), (cx2_pad, 0), (cx2_pad, 1)],
                   pad_writer(x2_pad, 3))
        bt = cvp.tile([128, 2, 28, 28], f16, tag="bt")

        def bt_writer(m, h, ps):
            o = bt[:, m, h * 14:(h + 1) * 14, :]
            if h == 0:
                nc.scalar.activation(o, ps, ACTF.Relu, bias=b_conv[:, 4, m:m+1])
            else:
                nc.vector.tensor_scalar(o, ps, b_conv[:, 4, m:m+1], 0.0,
                                        op0=ALU.add, op1=ALU.max)

        conv_block(4, [(x2_pad, 0), (x2_pad, 1)], bt_writer)
        btf = bt.rearrange("p m x y -> p m (x y)")

        # ---------------- cf mlp: z1 = gelu(cat @ W1 + b1); z = z1 @ W2 + b2
        z1 = tkp.tile([128, 4, 784], f16, tag="z1")
        for m in range(4):
            for h in range(2):
                ps = psp.tile([128, 392], f32, tag="ps")
                for kc in range(6):
                    rhs = (aT[:, kc, HSL[h]] if kc < 4
                           else btf[:, kc - 4, HSL[h]])
                    nc.tensor.matmul(ps, w_cf1[:, kc, m * 128:(m + 1) * 128], rhs,
                                     start=(kc == 0), stop=(kc == 5))
                nc.scalar.activation(z1[:, m, HSL[h]], ps, GELU,
                                     bias=b_cf1[:, m:m+1])
        z = tkp.tile([128, 4, 784], f32r, tag="z")
        for m in range(4):
            for h in range(2):
                ps = psp.tile([128, 392], f32, tag="ps")
                for kc in range(4):
                    nc.tensor.matmul(ps, w_cf2[:, kc, m * 128:(m + 1) * 128],
                                     z1[:, kc, HSL[h]],
                                     start=(kc == 0), stop=(kc == 3))
                nc.scalar.activation(z[:, m, HSL[h]], ps, ACTF.Identity,
                                     bias=b_cf2[:, m:m+1])

        # ---------------- layernorm helper (stats + normalize to fp16)
        def layer_norm(src, dst):
            rstd = rwp.tile([1, 2, 392], f32r, tag="rstd")
            m2 = rwp.tile([1, 2, 392], f32r, tag="m2")
            for h in range(2):
                psA = psr.tile([1, 392], f32, tag="psrow")
                psB = psr.tile([1, 392], f32, tag="psrow")
                for kc in range(4):
                    nc.tensor.matmul(psA, ones32_col, src[:, kc, HSL[h]],
                                     start=(kc == 0), stop=(kc == 3))
                for kc in range(4):
                    sqt = sqp.tile([128, 392], f32r, tag="sq")
                    nc.vector.tensor_tensor(sqt, src[:, kc, HSL[h]],
                                            src[:, kc, HSL[h]], ALU.mult)
                    nc.tensor.matmul(psB, ones32_col, sqt,
                                     start=(kc == 0), stop=(kc == 3))
                s1row = rwp.tile([1, 392], f32r, tag="s1row")
                nc.scalar.activation(s1row, psA, ACTF.Copy)
                t1 = rwp.tile([1, 392], f32r, tag="t1")
                nc.vector.tensor_tensor(t1, s1row, s1row, ALU.mult)
                nc.vector.tensor_scalar_mul(t1, t1, -1.0 / EMB)
                t2 = rwp.tile([1, 392], f32r, tag="t2")
                nc.vector.tensor_add(t2, psB, t1)
                sdev = rwp.tile([1, 392], f32r, tag="t1")
                nc.scalar.activation(sdev, t2, ACTF.Sqrt,
                                     bias=eps_row, scale=1.0 / EMB)
                nc.vector.reciprocal(rstd[:, h], sdev)
                nc.vector.tensor_tensor(m2[:, h], s1row, rstd[:, h], ALU.mult)
                nc.vector.tensor_scalar_mul(m2[:, h], m2[:, h], 1.0 / EMB)
            for h in range(2):
                psrd = psp.tile([128, 392], f32, tag="ps")
                nc.tensor.matmul(psrd, ones32_row, rstd[:, h], start=True, stop=True)
                psm2 = psp.tile([128, 392], f32, tag="ps")
                nc.tensor.matmul(psm2, ones32_row, m2[:, h], start=True, stop=True)
                for kc in range(4):
                    tmpt = sqp.tile([128, 392], f32r, tag="sq")
                    nc.vector.tensor_tensor(tmpt, src[:, kc, HSL[h]], psrd, ALU.mult)
                    nc.vector.tensor_tensor(dst[:, kc, HSL[h]], tmpt, psm2,
                                            ALU.subtract)

        zhat = tkp.tile([128, 4, 784], f16, tag="zhat")
        layer_norm(z, zhat)

        # ---------------- kqv: k,q feature-major; v token-major
        kfm = tkp.tile([128, 4, 784], f16, tag="kfm")
        qfm = tkp.tile([128, 4, 784], f16, tag="qfm")
        for mi in range(8):
            dstT, mloc = (kfm, mi) if mi < 4 else (qfm, mi - 4)
            for h in range(2):
                ps = psp.tile([128, 392], f32, tag="ps")
                for kc in range(4):
                    nc.tensor.matmul(ps, w_kqv[:, kc, mi * 128:(mi + 1) * 128],
                                     zhat[:, kc, HSL[h]],
                                     start=(kc == 0), stop=(kc == 3))
                nc.scalar.activation(dstT[:, mloc, HSL[h]], ps, ACTF.Identity,
                                     bias=b_kqv[:, mi:mi+1])
        vtok = tkp.tile([128, 7, 512], f16, tag="vtok")
        for ti in range(7):
            tcn = 128 if ti < 6 else 16
            tsl = slice(ti * 128, ti * 128 + tcn)
            ps = psp.tile([128, 512], f32, tag="ps")
            for kc in range(4):
                nc.tensor.matmul(ps[:tcn], zhat[:, kc, tsl], w_kqv[:, kc, 1024:1536],
                                 start=(kc == 0), stop=(kc == 3))
            nc.vector.tensor_add(vtok[:tcn, ti], ps[:tcn], w_c2vbc[:tcn])

        # ---------------- FAVOR+ feature maps
        def xd_rows(src, tag):
            xd = rwp.tile([1, 2, 392], f16, tag=tag)
            for h in range(2):
                psx = psr.tile([1, 392], f32, tag="psrow")
                for kc in range(4):
                    sqt = sqp.tile([128, 392], f16, tag="sqh")
                    nc.vector.tensor_tensor(sqt, src[:, kc, HSL[h]],
                                            src[:, kc, HSL[h]], ALU.mult)
                    nc.tensor.matmul(psx, neghalf_col, sqt,
                                     start=(kc == 0), stop=(kc == 3))
                nc.scalar.activation(xd[:, h], psx, ACTF.Copy)
            return xd

        xdk = xd_rows(kfm, "xdk")
        xdq = xd_rows(qfm, "xdq")
        kptok = tkp.tile([128, 7, 256], f16, tag="kptok")
        xdk_flat = xdk.rearrange("o h x -> o (h x)")
        for ti in range(7):
            tcn = 128 if ti < 6 else 16
            tsl = slice(ti * 128, ti * 128 + tcn)
            ps = psp.tile([128, 256], f32, tag="ps")
            for kc in range(4):
                nc.tensor.matmul(ps[:tcn], kfm[:, kc, tsl], w_wT[:, kc],
                                 start=(kc == 0), stop=False)
            nc.tensor.matmul(ps[:tcn], xdk_flat[:, tsl], ones16_row,
                             start=False, stop=True)
            nc.scalar.activation(kptok[:tcn, ti], ps[:tcn], ACTF.Exp,
                                 bias=negln16_col[:tcn])
        qp = tkp.tile([128, 2, 784], f16, tag="qp")
        for mi in range(2):
            for h in range(2):
                ps = psp.tile([128, 392], f32, tag="ps")
                for kc in range(4):
                    nc.tensor.matmul(ps, w_wT[:, kc, mi * 128:(mi + 1) * 128],
                                     qfm[:, kc, HSL[h]],
                                     start=(kc == 0), stop=False)
                nc.tensor.matmul(ps, ones16_row[:, 0:128], xdq[:, h],
                                 start=False, stop=True)
                nc.scalar.activation(qp[:, mi, HSL[h]], ps, ACTF.Exp,
                                     bias=negln16_col)

        # ---------------- ksum, D, kptv, y
        kscol = smp.tile([128, 2, 1], f16, tag="kscol")
        for mc in range(2):
            psk = psp.tile([128, 1], f32, tag="ps")
            for ti in range(7):
                tcn = 128 if ti < 6 else 16
                nc.tensor.matmul(psk, kptok[:tcn, ti, mc * 128:(mc + 1) * 128],
                                 ones16_col[:tcn], start=(ti == 0), stop=(ti == 6))
            nc.vector.tensor_copy(kscol[:, mc], psk)
        invd = rwp.tile([1, 2, 392], f32r, tag="invd")
        for h in range(2):
            psD = psr.tile([1, 392], f32, tag="psrow")
            for mc in range(2):
                nc.tensor.matmul(psD, kscol[:, mc], qp[:, mc, HSL[h]],
                                 start=(mc == 0), stop=(mc == 1))
            dreg = rwp.tile([1, 392], f32r, tag="t1")
            nc.vector.tensor_scalar_add(dreg, psD, 1e-8)
            nc.vector.reciprocal(invd[:, h], dreg)
        kptvT = tkp.tile([128, 2, 512], f16, tag="kptvT")
        for mc in range(2):
            ps = psp.tile([128, 512], f32, tag="ps")
            for ti in range(7):
                tcn = 128 if ti < 6 else 16
                nc.tensor.matmul(ps, kptok[:tcn, ti, mc * 128:(mc + 1) * 128],
                                 vtok[:tcn, ti], start=(ti == 0), stop=(ti == 6))
            nc.scalar.activation(kptvT[:, mc], ps, ACTF.Copy)
        y = tkp.tile([128, 4, 784], f16, tag="kfm")
        for h in range(2):
            psb = psp.tile([128, 392], f32, tag="ps")
            nc.tensor.matmul(psb, ones32_row, invd[:, h], start=True, stop=True)
            invb = sqp.tile([128, 392], f32r, tag="sq")
            nc.scalar.activation(invb, psb, ACTF.Copy)
            for vc in range(4):
                psy = psp.tile([128, 392], f32, tag="ps")
                for mc in range(2):
                    nc.tensor.matmul(psy, kptvT[:, mc, vc * 128:(vc + 1) * 128],
                                     qp[:, mc, HSL[h]],
                                     start=(mc == 0), stop=(mc == 1))
                nc.vector.tensor_tensor(y[:, vc, HSL[h]], psy, invb, ALU.mult)

        # ---------------- proj + residual into z
        for m in range(4):
            for h in range(2):
                ps = psp.tile([128, 392], f32, tag="ps")
                for kc in range(4):
                    nc.tensor.matmul(ps, w_proj[:, kc, m * 128:(m + 1) * 128],
                                     y[:, kc, HSL[h]],
                                     start=(kc == 0), stop=(kc == 3))
                tmp = sqp.tile([128, 392], f32r, tag="sq")
                nc.scalar.activation(tmp, ps, ACTF.Identity, bias=b_proj[:, m:m+1])
                nc.vector.tensor_add(z[:, m, HSL[h]], z[:, m, HSL[h]], tmp)

        # ---------------- LN2 + mlp
        zhat2 = tkp.tile([128, 4, 784], f16, tag="zhat")
        layer_norm(z, zhat2)
        g1t = tkp.tile([128, 4, 784], f16, tag="z1")
        for m in range(4):
            for h in range(2):
                ps = psp.tile([128, 392], f32, tag="ps")
                for kc in range(4):
                    nc.tensor.matmul(ps, w_mlp1[:, kc, m * 128:(m + 1) * 128],
                                     zhat2[:, kc, HSL[h]],
                                     start=(kc == 0), stop=(kc == 3))
                nc.scalar.activation(g1t[:, m, HSL[h]], ps, GELU,
                                     bias=b_mlp1[:, m:m+1])
        # mlp2 token-major + identity residual; write output
        for ti in range(7):
            tcn = 128 if ti < 6 else 16
            tsl = slice(ti * 128, ti * 128 + tcn)
            ps = psp.tile([128, 512], f32, tag="ps")
            for kc in range(4):
                nc.tensor.matmul(ps[:tcn], g1t[:, kc, tsl], w_mlp2[:, kc],
                                 start=(kc == 0), stop=False)
            for kc in range(4):
                nc.tensor.matmul(ps[:tcn], z[:, kc, tsl], w_i512[:, kc],
                                 start=False, stop=(kc == 3))
            osb = otp.tile([128, 512], f32, tag="osb")
            nc.vector.tensor_add(osb[:tcn], ps[:tcn], w_b2bc[:tcn])
            dma(d["d_out"][it, ti * 128: ti * 128 + tcn, :], osb[:tcn])


# ---------------------------------------------------------------- entry
_BUILD_CACHE = {}


def _get_nc(n_items):
    if n_items not in _BUILD_CACHE:
        _BUILD_CACHE[n_items] = _build_bass(n_items)
    return _BUILD_CACHE[n_items]


def kernel(a, b, c, params, w):
    from concourse.bass_utils import run_bass_kernel_spmd

    arrs = _prep_arrays(a, b, c, params, w)
    nc = _get_nc(ITEMS)

    shared = {k: v for k, v in arrs.items() if k not in ("a_t", "b_in", "c_in")}
    in_maps = []
    for core in range(N_CORES):
        s = slice(core * ITEMS, (core + 1) * ITEMS)
        m = dict(shared)
        m["a_t"] = arrs["a_t"][s]
        m["b_in"] = arrs["b_in"][s]
        m["c_in"] = arrs["c_in"][s]
        in_maps.append(m)

    res = run_bass_kernel_spmd(nc, in_maps, core_ids=list(range(N_CORES)))
    global LAST_RESULT
    LAST_RESULT = res
    out = np.concatenate([r["out"] for r in res.results], axis=0)
    return out.astype(np.float32)


LAST_RESULT = None


# revision 46
# speedup vs baseline: 1.0382x; 1.0382x over previous
"""Trainium2 Bass kernel for nn_FEM_33251636806316 (sparse_attention).

Data-parallel over batch: 64 items -> 8 NeuronCores x 8 items each.
All heavy matmuls in fp16 (full PE rate), psum fp32, residual stream and
LN statistics in float32r (fp32 storage, full-rate matmul for N>=256).
"""

import math

import numpy as np

# ---------------------------------------------------------------- constants
B = 64
HW = 14
EMB = 512
CUR = 1024
CH = 256  # conv channels
M = 256  # FAVOR+ features
T = 4 * HW * HW  # 784 tokens
N_CORES = 8
ITEMS = B // N_CORES  # 8 per core
NEG_LN16 = -math.log(math.sqrt(M))  # -ln(16)

# conv layer chunk table: (chunk_base, n_in_chunks) in the packed conv weight
_CONV_CHUNKS = [(0, 4), (4, 4), (8, 4), (12, 4), (16, 2)]

# debug: replace Gelu by Identity (CoreSim has no Gelu model)
GELU_IDENTITY = False


# ---------------------------------------------------------------- host prep
def _prep_arrays(a, b, c, params, w):
    """Host-side marshalling: weight folding, layout transforms, fp16 casts."""
    p = params
    f16 = np.float16
    f32 = np.float32

    def to16(x):
        return np.ascontiguousarray(np.asarray(x), dtype=f16)

    def to32(x):
        return np.ascontiguousarray(np.asarray(x), dtype=f32)

    arrs = {}
    # inputs
    arrs["a_t"] = to16(np.asarray(a).transpose(0, 2, 1))  # [B, 512, 784]
    arrs["b_in"] = to32(b)  # [B, 196, 1024]
    arrs["c_in"] = to32(np.asarray(c).reshape(B, CH, 28 * 28))  # [B, 256, 784]

    # conv weights: fold BN scale into W, transpose to [ic, tap, oc], chunked
    chunks = []
    bias = np.zeros((5, CH), f32)
    for i in range(1, 6):
        wi = np.asarray(p[f"cbr{i}_w"], f32)  # [oc, ic, 3, 3]
        si = np.asarray(p[f"cbr{i}_s"], f32)
        bi = np.asarray(p[f"cbr{i}_b"], f32)
        wi = wi * si[:, None, None, None]
        bias[i - 1] = bi
        wt = wi.transpose(1, 2, 3, 0).reshape(wi.shape[1], 9, CH)  # [ic,9,oc]
        nch = wi.shape[1] // 128
        chunks.append(wt.reshape(nch, 128, 9, CH))
    arrs["convw"] = to16(np.concatenate(chunks, 0))  # [18, 128, 9, 256]
    arrs["convb"] = bias  # [5, 256]

    arrs["cf1w"] = to16(p["cf_w1"])  # [768, 512]
    arrs["cf1b"] = to32(p["cf_b1"])
    arrs["cf2w"] = to16(p["cf_w2"])
    arrs["cf2b"] = to32(p["cf_b2"])

    # LN1 folded into kqv
    g1 = np.asarray(p["ln1_g"], f32)
    b1 = np.asarray(p["ln1_b"], f32)
    kqvw = np.asarray(p["kqv_w"], f32)
    kqvb = np.asarray(p["kqv_b"], f32)
    arrs["kqvw"] = to16(kqvw * g1[:, None])  # [512, 1536]
    c2 = b1 @ kqvw + kqvb  # [1536]
    arrs["kqvc2"] = to32(c2[:1024])  # k,q biases
    arrs["c2vbc"] = to16(np.tile(c2[1024:1536][None, :], (128, 1)))  # [128,512]

    arrs["wT"] = to16(np.asarray(w, f32).T)  # [512, 256]
    arrs["projw"] = to16(p["proj_w"])
    arrs["projb"] = to32(p["proj_b"])

    # LN2 folded into mlp1
    g2 = np.asarray(p["ln2_g"], f32)
    b2 = np.asarray(p["ln2_b"], f32)
    m1w = np.asarray(p["mlp_w1"], f32)
    arrs["mlp1w"] = to16(m1w * g2[:, None])
    arrs["mlp1c2"] = to32(b2 @ m1w + np.asarray(p["mlp_b1"], f32))
    arrs["mlp2w"] = to16(p["mlp_w2"])
    arrs["b2bc"] = to32(np.tile(np.asarray(p["mlp_b2"], f32)[None, :], (128, 1)))

    arrs["i512"] = np.eye(512, dtype=f32)

    arrs["caw1"] = to16(p["ca_w1"])  # [256, 16]
    arrs["caw2"] = to16(p["ca_w2"])  # [16, 256]

    # spatial-attention banded matrices: band[ch*7+kx][y_in, y_out]
    saw = np.asarray(p["sa_w"], f32)[0]  # [2, 7, 7]
    bands = np.zeros((14, 28, 28), f32)
    for ch in range(2):
        for kx in range(7):
            for yo in range(28):
                for yi in range(max(0, yo - 3), min(28, yo + 4)):
                    bands[ch * 7 + kx, yi, yo] = saw[ch, yi - yo + 3, kx]
    bands[0:7] *= 1.0 / CH  # fold the channel-mean 1/256 into ch-0 bands
    arrs["bands"] = to16(bands)
    return arrs


# ---------------------------------------------------------------- bass build
def _build_bass(n_items):
    from contextlib import ExitStack

    import concourse.bass as bass
    import concourse.tile as tile
    from concourse import bacc, mybir
    from concourse.masks import make_identity

    f16 = mybir.dt.float16
    f32 = mybir.dt.float32
    f32r = mybir.dt.float32r
    AX = mybir.AxisListType
    ALU = mybir.AluOpType
    ACTF = mybir.ActivationFunctionType

    nc = bacc.Bacc("TRN2", target_bir_lowering=False, debug=False)

    # ---- dram tensors
    d_a = nc.dram_tensor("a_t", [n_items, EMB, T], f16, kind="ExternalInput").ap()
    d_b = nc.dram_tensor("b_in", [n_items, 196, CUR], f32, kind="ExternalInput").ap()
    d_c = nc.dram_tensor("c_in", [n_items, CH, 784], f32, kind="ExternalInput").ap()
    d_convw = nc.dram_tensor("convw", [18, 128, 9, CH], f16, kind="ExternalInput").ap()
    d_convb = nc.dram_tensor("convb", [5, CH], f32, kind="ExternalInput").ap()
    d_cf1w = nc.dram_tensor("cf1w", [768, 512], f16, kind="ExternalInput").ap()
    d_cf1b = nc.dram_tensor("cf1b", [512], f32, kind="ExternalInput").ap()
    d_cf2w = nc.dram_tensor("cf2w", [512, 512], f16, kind="ExternalInput").ap()
    d_cf2b = nc.dram_tensor("cf2b", [512], f32, kind="ExternalInput").ap()
    d_kqvw = nc.dram_tensor("kqvw", [512, 1536], f16, kind="ExternalInput").ap()
    d_kqvc2 = nc.dram_tensor("kqvc2", [1024], f32, kind="ExternalInput").ap()
    d_c2vbc = nc.dram_tensor("c2vbc", [128, 512], f16, kind="ExternalInput").ap()
    d_wT = nc.dram_tensor("wT", [512, 256], f16, kind="ExternalInput").ap()
    d_projw = nc.dram_tensor("projw", [512, 512], f16, kind="ExternalInput").ap()
    d_projb = nc.dram_tensor("projb", [512], f32, kind="ExternalInput").ap()
    d_mlp1w = nc.dram_tensor("mlp1w", [512, 512], f16, kind="ExternalInput").ap()
    d_mlp1c2 = nc.dram_tensor("mlp1c2", [512], f32, kind="ExternalInput").ap()
    d_mlp2w = nc.dram_tensor("mlp2w", [512, 512], f16, kind="ExternalInput").ap()
    d_b2bc = nc.dram_tensor("b2bc", [128, 512], f32, kind="ExternalInput").ap()
    d_i512 = nc.dram_tensor("i512", [512, 512], f32r, kind="ExternalInput").ap()
    d_caw1 = nc.dram_tensor("caw1", [CH, 16], f16, kind="ExternalInput").ap()
    d_caw2 = nc.dram_tensor("caw2", [16, CH], f16, kind="ExternalInput").ap()
    d_bands = nc.dram_tensor("bands", [14, 28, 28], f16, kind="ExternalInput").ap()
    d_out = nc.dram_tensor("out", [n_items, T, EMB], f32, kind="ExternalOutput").ap()

    with tile.TileContext(nc) as tc:
        with ExitStack() as ctx:
            with nc.allow_low_precision(reason="intentional fp16 matmul pipeline"):
                _emit(tc, ctx, nc, n_items, locals())
    nc.compile()
    return nc


def _emit(tc, ctx, nc, n_items, d):
    import concourse.bass as bass
    import concourse.bass_isa as bass_isa
    from concourse import mybir
    from concourse.masks import make_identity

    f16 = mybir.dt.float16
    f32 = mybir.dt.float32
    f32r = mybir.dt.float32r
    AX = mybir.AxisListType
    ALU = mybir.AluOpType
    ACTF = mybir.ActivationFunctionType
    GELU = ACTF.Identity if GELU_IDENTITY else ACTF.Gelu

    pool = lambda name, bufs: ctx.enter_context(tc.tile_pool(name=name, bufs=bufs))
    wp = pool("weights", 1)  # resident weights
    cwp = pool("convw", 2)  # streamed conv weights
    cvp = pool("convact", 1)  # conv activations
    stg = pool("stage", 1)  # input staging
    tkp = pool("tok", 1)  # token activations
    sqp = pool("sq", 2)  # small square/tmp tiles
    rwp = pool("rows", 1)  # [1, x] rows
    smp = pool("small", 1)  # small per-item tensors
    otp = pool("outsb", 2)  # output staging
    psp = ctx.enter_context(tc.tile_pool(name="psum", bufs=3, space="PSUM"))
    psr = ctx.enter_context(tc.tile_pool(name="psrow", bufs=2, space="PSUM"))

    dma = nc.sync.dma_start

    # ---------------- resident weights
    w_cf1 = wp.tile([128, 6, 512], f16, tag="w_cf1")
    dma(w_cf1, d["d_cf1w"].rearrange("(c p) n -> p c n", p=128))
    w_cf2 = wp.tile([128, 4, 512], f16, tag="w_cf2")
    dma(w_cf2, d["d_cf2w"].rearrange("(c p) n -> p c n", p=128))
    w_kqv = wp.tile([128, 4, 1536], f16, tag="w_kqv")
    dma(w_kqv, d["d_kqvw"].rearrange("(c p) n -> p c n", p=128))
    w_wT = wp.tile([128, 4, 256], f16, tag="w_wT")
    dma(w_wT, d["d_wT"].rearrange("(c p) n -> p c n", p=128))
    w_proj = wp.tile([128, 4, 512], f16, tag="w_proj")
    dma(w_proj, d["d_projw"].rearrange("(c p) n -> p c n", p=128))
    w_mlp1 = wp.tile([128, 4, 512], f16, tag="w_mlp1")
    dma(w_mlp1, d["d_mlp1w"].rearrange("(c p) n -> p c n", p=128))
    w_mlp2 = wp.tile([128, 4, 512], f16, tag="w_mlp2")
    dma(w_mlp2, d["d_mlp2w"].rearrange("(c p) n -> p c n", p=128))
    w_i512 = wp.tile([128, 4, 512], f32r, tag="w_i512")
    dma(w_i512, d["d_i512"].rearrange("(c p) n -> p c n", p=128))
    w_c2vbc = wp.tile([128, 512], f16, tag="w_c2vbc")
    dma(w_c2vbc, d["d_c2vbc"])
    w_b2bc = wp.tile([128, 512], f32, tag="w_b2bc")
    dma(w_b2bc, d["d_b2bc"])
    w_caw1 = wp.tile([128, 2, 16], f16, tag="w_caw1")
    dma(w_caw1, d["d_caw1"].rearrange("(c p) n -> p c n", p=128))
    w_caw2 = wp.tile([16, 256], f16, tag="w_caw2")
    dma(w_caw2, d["d_caw2"])
    w_bands = wp.tile([28, 14, 28], f16, tag="w_bands")
    dma(w_bands, d["d_bands"].rearrange("i p j -> p i j"))
    b_conv = wp.tile([128, 5, 2], f32, tag="b_conv")
    dma(b_conv, d["d_convb"].rearrange("l (m p) -> p l m", p=128))
    b_cf1 = wp.tile([128, 4], f32, tag="b_cf1")
    dma(b_cf1, d["d_cf1b"].rearrange("(m p) -> p m", p=128))
    b_cf2 = wp.tile([128, 4], f32, tag="b_cf2")
    dma(b_cf2, d["d_cf2b"].rearrange("(m p) -> p m", p=128))
    b_kqv = wp.tile([128, 8], f32, tag="b_kqv")
    dma(b_kqv, d["d_kqvc2"].rearrange("(m p) -> p m", p=128))
    b_proj = wp.tile([128, 4], f32, tag="b_proj")
    dma(b_proj, d["d_projb"].rearrange("(m p) -> p m", p=128))
    b_mlp1 = wp.tile([128, 4], f32, tag="b_mlp1")
    dma(b_mlp1, d["d_mlp1c2"].rearrange("(m p) -> p m", p=128))

    ident = wp.tile([128, 128], f16, tag="ident")
    make_identity(nc, ident)
    ones16_col = wp.tile([128, 1], f16, tag="ones16_col")
    nc.gpsimd.memset(ones16_col, 1.0)
    neghalf_col = wp.tile([128, 1], f16, tag="neghalf_col")
    nc.gpsimd.memset(neghalf_col, -0.5)
    ones16_row = wp.tile([1, 256], f16, tag="ones16_row")
    nc.gpsimd.memset(ones16_row, 1.0)
    ones32_col_f = wp.tile([128, 1], f32, tag="ones32_col")
    nc.gpsimd.memset(ones32_col_f, 1.0)
    ones32_col = ones32_col_f.bitcast(f32r)
    ones32_row_f = wp.tile([1, 128], f32, tag="ones32_row")
    nc.gpsimd.memset(ones32_row_f, 1.0)
    ones32_row = ones32_row_f.bitcast(f32r)
    eps_row = wp.tile([1, 1], f32, tag="eps_row")
    nc.gpsimd.memset(eps_row, 1e-5)
    eps8_row = wp.tile([1, 1], f32, tag="eps8_row")
    nc.gpsimd.memset(eps8_row, 1e-8)
    # persistent padded conv tiles: borders zeroed once, interiors rewritten
    t_pad = wp.tile([128, 2, 30, 30], f16, tag="t_pad")
    nc.gpsimd.memset(t_pad, 0.0)
    c_pad = wp.tile([128, 2, 30, 30], f16, tag="c_pad")
    nc.gpsimd.memset(c_pad, 0.0)
    tx_pad = wp.tile([128, 2, 30, 30], f16, tag="tx_pad")
    nc.gpsimd.memset(tx_pad, 0.0)
    cx_pad = wp.tile([128, 2, 30, 30], f16, tag="cx_pad")
    nc.gpsimd.memset(cx_pad, 0.0)
    x1_pad = wp.tile([128, 2, 30, 30], f16, tag="x1_pad")
    nc.gpsimd.memset(x1_pad, 0.0)
    sa_t = wp.tile([28, 2, 34], f16, tag="sa_t")
    nc.gpsimd.memset(sa_t, 0.0)
    negln16_col = wp.tile([128, 1], f32, tag="negln16_col")
    nc.gpsimd.memset(negln16_col, NEG_LN16)

    HSL = [slice(0, 392), slice(392, 784)]  # token halves

    # =============================================================== items
    for it in range(n_items):
        # ---------------- input loads
        tb0 = stg.tile([128, 1024], f32, tag="bstage")
        dma(tb0, d["d_b"][it, 0:128, :])
        tc0 = stg.tile([128, 2, 784], f32, tag="c0")
        dma(tc0, d["d_c"][it].rearrange("(cj p) x -> p cj x", p=128))
        aT = stg.tile([128, 4, 784], f16, tag="aT")
        dma(aT, d["d_a"][it].rearrange("(cj p) t -> p cj t", p=128))

        # ---------------- pixel shuffle b -> t_pad
        b16_0 = stg.tile([128, 1024], f16, tag="b16_0")
        nc.scalar.activation(b16_0, tb0, ACTF.Copy)
        tb1 = stg.tile([128, 1024], f32, tag="bstage")
        dma(tb1[:68], d["d_b"][it, 128:196, :])
        b16_1 = stg.tile([68, 1024], f16, tag="b16_1")
        nc.scalar.activation(b16_1, tb1[:68], ACTF.Copy)
        bv0 = b16_0.rearrange("p (c g) -> p g c", g=4)
        bv1 = b16_1.rearrange("p (c g) -> p g c", g=4)
        for cj in range(2):
            for g in range(4):
                d1, d2 = g // 2, g % 2
                ps = psp.tile([128, 196], f16, tag="ps16")
                nc.tensor.transpose(
                    ps[:, 0:128], bv0[:, g, cj * 128:(cj + 1) * 128], ident)
                nc.tensor.transpose(
                    ps[:, 128:196], bv1[:, g, cj * 128:(cj + 1) * 128],
                    ident[:68, :68])
                dst = t_pad[:, cj, 1:29, 1:29].rearrange(
                    "p (i a) (j b) -> p a b i j", a=2, b=2)[:, d1, d2]
                nc.scalar.activation(dst, ps.rearrange("p (i j) -> p i j", j=14),
                                     ACTF.Copy)

        # ---------------- c load/pad + channel attention
        for cj in range(2):
            nc.scalar.activation(
                c_pad[:, cj, 1:29, 1:29],
                tc0[:, cj].rearrange("p (x y) -> p x y", y=28), ACTF.Copy)
        cred = smp.tile([128, 2, 2], f32, tag="cred")
        for cj in range(2):
            nc.vector.reduce_sum(cred[:, cj, 0:1], tc0[:, cj], axis=AX.X)
            nc.vector.reduce_max(cred[:, cj, 1:2], tc0[:, cj], axis=AX.X)
        v2 = smp.tile([128, 2, 2], f16, tag="v2")
        for cj in range(2):
            nc.vector.tensor_scalar_mul(v2[:, cj, 0:1], cred[:, cj, 0:1], 1.0 / 784)
            nc.vector.tensor_copy(v2[:, cj, 1:2], cred[:, cj, 1:2])
        psh = psp.tile([16, 2], f32, tag="ps")
        nc.tensor.matmul(psh, w_caw1[:, 0], v2[:, 0], start=True, stop=False)
        nc.tensor.matmul(psh, w_caw1[:, 1], v2[:, 1], start=False, stop=True)
        h16 = smp.tile([16, 2], f16, tag="h16")
        nc.scalar.activation(h16, psh, ACTF.Relu)
        cc = smp.tile([128, 2, 1], f32, tag="cc")
        for cj in range(2):
            ps2 = psp.tile([128, 1], f32, tag="ps")
            nc.tensor.matmul(ps2, w_caw2[:, cj * 128:(cj + 1) * 128], h16[:, 0:1],
                             start=True, stop=False)
            nc.tensor.matmul(ps2, w_caw2[:, cj * 128:(cj + 1) * 128], h16[:, 1:2],
                             start=False, stop=True)
            cct = smp.tile([128, 1], f32, tag="cct")
            nc.scalar.activation(cct, ps2, ACTF.Tanh, scale=0.5)
            nc.vector.tensor_scalar(cc[:, cj], cct, 0.5, 0.5,
                                    op0=ALU.mult, op1=ALU.add)

        # ---------------- spatial attention
        tadd = smp.tile([128, 28, 28], f16, tag="tadd")
        nc.vector.tensor_add(
            tadd, t_pad[:, 0, 1:29, 1:29], t_pad[:, 1, 1:29, 1:29])
        tmax = smp.tile([128, 28, 28], f16, tag="tmax")
        nc.vector.tensor_tensor(
            tmax, t_pad[:, 0, 1:29, 1:29], t_pad[:, 1, 1:29, 1:29], ALU.max)
        maxall = smp.tile([128, 28, 28], f16, tag="maxall")
        nc.gpsimd.partition_all_reduce(maxall, tmax, 128, bass_isa.ReduceOp.max)
        maxrow = maxall[0:1]
        meanrow = rwp.tile([1, 2, 392], f16, tag="meanrow")
        for h in range(2):
            psm = psr.tile([1, 14, 28], f32, tag="psrow")
            nc.tensor.matmul(
                psm, ones16_col, tadd[:, h * 14:(h + 1) * 14, :],
                start=True, stop=True)
            nc.scalar.activation(meanrow[:, h], psm.rearrange("o i j -> o (i j)"),
                                 ACTF.Copy)
        dma(sa_t[:, 0, 3:31], meanrow.rearrange("o h x -> o (h x)"))
        dma(sa_t[:, 1, 3:31], maxrow.rearrange("o x y -> o (x y)"))
        pssa = psp.tile([28, 28], f32, tag="ps")
        for ch in range(2):
            for kx in range(7):
                nc.tensor.matmul(
                    pssa, w_bands[:, ch * 7 + kx, :], sa_t[:28, ch, kx:kx + 28],
                    start=(ch == 0 and kx == 0), stop=(ch == 1 and kx == 6))
        ts28 = smp.tile([28, 28], f16, tag="ts28")
        tst = smp.tile([28, 28], f32, tag="tst")
        nc.scalar.activation(tst, pssa, ACTF.Tanh, scale=0.5)
        nc.vector.tensor_scalar(ts28, tst, 0.5, 0.5, op0=ALU.mult, op1=ALU.add)
        tsrow = rwp.tile([1, 784], f16, tag="tsrow")
        dma(tsrow, ts28.rearrange("p x -> p x"))

        # ---------------- t_x = t * cc, c_x = c * ts
        for cj in range(2):
            nc.vector.tensor_scalar_mul(tx_pad[:, cj], t_pad[:, cj], cc[:, cj])
        for h in range(2):
            psts = psp.tile([128, 14, 28], f32, tag="ps")
            nc.tensor.matmul(
                psts, ones16_row[:, 0:128],
                tsrow[:, h * 392:(h + 1) * 392].rearrange("o (i j) -> o i j", j=28),
                start=True, stop=True)
            for cj in range(2):
                r0 = 1 + h * 14
                nc.vector.tensor_tensor(
                    cx_pad[:, cj, r0:r0 + 14, 1:29],
                    c_pad[:, cj, r0:r0 + 14, 1:29], psts, ALU.mult)

        # ---------------- conv blocks
        def conv_block(lidx, in_specs, out_write):
            cb, nch = _CONV_CHUNKS[lidx]
            wt = []
            for half in range((nch + 1) // 2):
                cw = cwp.tile([128, 2, 9, CH], f16, tag="cw")
                n = min(2, nch - half * 2)
                dma(cw[:, 0:n],
                    d["d_convw"][cb + half * 2: cb + half * 2 + n].rearrange(
                        "c p t o -> p c t o"))
                wt.append(cw)
            for m in range(2):
                for h in range(2):
                    ps = psp.tile([128, 14, 28], f32, tag="ps")
                    idx = 0
                    last = nch * 9 - 1
                    for kc in range(nch):
                        tile_in, cj = in_specs[kc]
                        for dy in range(3):
                            for dx in range(3):
                                lhsT = wt[kc // 2][:, kc % 2, dy * 3 + dx,
                                                   m * 128:(m + 1) * 128]
                                rhs = tile_in[:, cj, h * 14 + dy: h * 14 + dy + 14,
                                              dx:dx + 28]
                                nc.tensor.matmul(ps, lhsT, rhs,
                                                 start=(idx == 0), stop=(idx == last))
                                idx += 1
                    out_write(m, h, ps)

        def pad_writer(dst, lidx):
            def write(m, h, ps):
                o = dst[:, m, 1 + h * 14: 15 + h * 14, 1:29]
                if h == 0:
                    nc.scalar.activation(o, ps, ACTF.Relu, bias=b_conv[:, lidx, m:m+1])
                else:
                    nc.vector.tensor_scalar(o, ps, b_conv[:, lidx, m:m+1], 0.0,
                                            op0=ALU.add, op1=ALU.max)
            return write

        conv_block(0, [(tx_pad, 0), (tx_pad, 1), (cx_pad, 0), (cx_pad, 1)],
                   pad_writer(x1_pad, 0))
        u_pad = tx_pad
        conv_block(1, [(t_pad, 0), (t_pad, 1), (x1_pad, 0), (x1_pad, 1)],
                   pad_writer(u_pad, 1))
        cx2_pad = t_pad
        conv_block(2, [(c_pad, 0), (c_pad, 1), (x1_pad, 0), (x1_pad, 1)],
                   pad_writer(cx2_pad, 2))
        x2_pad = cx_pad
        conv_block(3, [(u_pad, 0), (u_pad, 1), (cx2_pad, 0), (cx2_pad, 1)],
                   pad_writer(x2_pad, 3))
        bt = cvp.tile([128, 2, 28, 28], f16, tag="bt")

        def bt_writer(m, h, ps):
            o = bt[:, m, h * 14:(h + 1) * 14, :]
            if h == 0:
                nc.scalar.activation(o, ps, ACTF.Relu, bias=b_conv[:, 4, m:m+1])
            else:
                nc.vector.tensor_scalar(o, ps, b_conv[:, 4, m:m+1], 0.0,
                                        op0=ALU.add, op1=ALU.max)

        conv_block(4, [(x2_pad, 0), (x2_pad, 1)], bt_writer)
        btf = bt.rearrange("p m x y -> p m (x y)")

        # ---------------- cf mlp: z1 = gelu(cat @ W1 + b1); z = z1 @ W2 + b2
        z1 = tkp.tile([128, 4, 784], f16, tag="z1")
        for m in range(4):
            for h in range(2):
                ps = psp.tile([128, 392], f32, tag="ps")
                for kc in range(6):
                    rhs = (aT[:, kc, HSL[h]] if kc < 4
                           else btf[:, kc - 4, HSL[h]])
                    nc.tensor.matmul(ps, w_cf1[:, kc, m * 128:(m + 1) * 128], rhs,
                                     start=(kc == 0), stop=(kc == 5))
                nc.scalar.activation(z1[:, m, HSL[h]], ps, GELU,
                                     bias=b_cf1[:, m:m+1])
        z = tkp.tile([128, 4, 784], f32r, tag="z")
        for m in range(4):
            for h in range(2):
                ps = psp.tile([128, 392], f32, tag="ps")
                for kc in range(4):
                    nc.tensor.matmul(ps, w_cf2[:, kc, m * 128:(m + 1) * 128],
                                     z1[:, kc, HSL[h]],
                                     start=(kc == 0), stop=(kc == 3))
                nc.scalar.activation(z[:, m, HSL[h]], ps, ACTF.Identity,
                                     bias=b_cf2[:, m:m+1])

        # ---------------- layernorm helper (stats + normalize to fp16)
        def layer_norm(src, dst):
            rstd = rwp.tile([1, 2, 392], f32r, tag="rstd")
            m2 = rwp.tile([1, 2, 392], f32r, tag="m2")
            for h in range(2):
                psA = psr.tile([1, 392], f32, tag="psrow")
                psB = psr.tile([1, 392], f32, tag="psrow")
                for kc in range(4):
                    nc.tensor.matmul(psA, ones32_col, src[:, kc, HSL[h]],
                                     start=(kc == 0), stop=(kc == 3))
                for kc in range(4):
                    sqt = sqp.tile([128, 392], f32r, tag="sq")
                    nc.vector.tensor_tensor(sqt, src[:, kc, HSL[h]],
                                            src[:, kc, HSL[h]], ALU.mult)
                    nc.tensor.matmul(psB, ones32_col, sqt,
                                     start=(kc == 0), stop=(kc == 3))
                s1row = rwp.tile([1, 392], f32r, tag="s1row")
                nc.scalar.activation(s1row, psA, ACTF.Copy)
                t1 = rwp.tile([1, 392], f32r, tag="t1")
                nc.vector.tensor_tensor(t1, s1row, s1row, ALU.mult)
                nc.vector.tensor_scalar_mul(t1, t1, -1.0 / EMB)
                t2 = rwp.tile([1, 392], f32r, tag="t2")
                nc.vector.tensor_add(t2, psB, t1)
                if GELU_IDENTITY:  # CoreSim lacks Abs_reciprocal_sqrt
                    sdev = rwp.tile([1, 392], f32r, tag="t1")
                    nc.scalar.activation(sdev, t2, ACTF.Sqrt,
                                         bias=eps_row, scale=1.0 / EMB)
                    nc.vector.reciprocal(rstd[:, h], sdev)
                else:
                    nc.scalar.activation(rstd[:, h], t2, ACTF.Abs_reciprocal_sqrt,
                                         bias=eps_row, scale=1.0 / EMB)
                nc.vector.tensor_tensor(m2[:, h], s1row, rstd[:, h], ALU.mult)
                nc.vector.tensor_scalar_mul(m2[:, h], m2[:, h], 1.0 / EMB)
            for h in range(2):
                psrd = psp.tile([128, 392], f32, tag="ps")
                nc.tensor.matmul(psrd, ones32_row, rstd[:, h], start=True, stop=True)
                psm2 = psp.tile([128, 392], f32, tag="ps")
                nc.tensor.matmul(psm2, ones32_row, m2[:, h], start=True, stop=True)
                for kc in range(4):
                    tmpt = sqp.tile([128, 392], f32r, tag="sq")
                    nc.vector.tensor_tensor(tmpt, src[:, kc, HSL[h]], psrd, ALU.mult)
                    nc.vector.tensor_tensor(dst[:, kc, HSL[h]], tmpt, psm2,
                                            ALU.subtract)

        zhat = tkp.tile([128, 4, 784], f16, tag="zhat")
        layer_norm(z, zhat)

        # ---------------- kqv: k,q feature-major; v token-major
        kfm = tkp.tile([128, 4, 784], f16, tag="kfm")
        qfm = tkp.tile([128, 4, 784], f16, tag="qfm")
        for mi in range(8):
            dstT, mloc = (kfm, mi) if mi < 4 else (qfm, mi - 4)
            for h in range(2):
                ps = psp.tile([128, 392], f32, tag="ps")
                for kc in range(4):
                    nc.tensor.matmul(ps, w_kqv[:, kc, mi * 128:(mi + 1) * 128],
                                     zhat[:, kc, HSL[h]],
                                     start=(kc == 0), stop=(kc == 3))
                nc.scalar.activation(dstT[:, mloc, HSL[h]], ps, ACTF.Identity,
                                     bias=b_kqv[:, mi:mi+1])
        vtok = tkp.tile([128, 7, 512], f16, tag="vtok")
        for ti in range(7):
            tcn = 128 if ti < 6 else 16
            tsl = slice(ti * 128, ti * 128 + tcn)
            ps = psp.tile([128, 512], f32, tag="ps")
            for kc in range(4):
                nc.tensor.matmul(ps[:tcn], zhat[:, kc, tsl], w_kqv[:, kc, 1024:1536],
                                 start=(kc == 0), stop=(kc == 3))
            nc.vector.tensor_add(vtok[:tcn, ti], ps[:tcn], w_c2vbc[:tcn])

        # ---------------- FAVOR+ feature maps
        def xd_rows(src, tag):
            xd = rwp.tile([1, 2, 392], f16, tag=tag)
            for h in range(2):
                psx = psr.tile([1, 392], f32, tag="psrow")
                for kc in range(4):
                    sqt = sqp.tile([128, 392], f16, tag="sqh")
                    nc.vector.tensor_tensor(sqt, src[:, kc, HSL[h]],
                                            src[:, kc, HSL[h]], ALU.mult)
                    nc.tensor.matmul(psx, neghalf_col, sqt,
                                     start=(kc == 0), stop=(kc == 3))
                nc.scalar.activation(xd[:, h], psx, ACTF.Copy)
            return xd

        xdk = xd_rows(kfm, "xdk")
        xdq = xd_rows(qfm, "xdq")
        kptok = tkp.tile([128, 7, 256], f16, tag="kptok")
        xdk_flat = xdk.rearrange("o h x -> o (h x)")
        for ti in range(7):
            tcn = 128 if ti < 6 else 16
            tsl = slice(ti * 128, ti * 128 + tcn)
            ps = psp.tile([128, 256], f32, tag="ps")
            for kc in range(4):
                nc.tensor.matmul(ps[:tcn], kfm[:, kc, tsl], w_wT[:, kc],
                                 start=(kc == 0), stop=False)
            nc.tensor.matmul(ps[:tcn], xdk_flat[:, tsl], ones16_row,
                             start=False, stop=True)
            nc.scalar.activation(kptok[:tcn, ti], ps[:tcn], ACTF.Exp,
                                 bias=negln16_col[:tcn])
        qp = tkp.tile([128, 2, 784], f16, tag="qp")
        for mi in range(2):
            for h in range(2):
                ps = psp.tile([128, 392], f32, tag="ps")
                for kc in range(4):
                    nc.tensor.matmul(ps, w_wT[:, kc, mi * 128:(mi + 1) * 128],
                                     qfm[:, kc, HSL[h]],
                                     start=(kc == 0), stop=False)
                nc.tensor.matmul(ps, ones16_row[:, 0:128], xdq[:, h],
                                 start=False, stop=True)
                nc.scalar.activation(qp[:, mi, HSL[h]], ps, ACTF.Exp,
                                     bias=negln16_col)

        # ---------------- ksum, D, kptv, y
        kscol = smp.tile([128, 2, 1], f16, tag="kscol")
        for mc in range(2):
            psk = psp.tile([128, 1], f32, tag="ps")
            for ti in range(7):
                tcn = 128 if ti < 6 else 16
                nc.tensor.matmul(psk, kptok[:tcn, ti, mc * 128:(mc + 1) * 128],
                                 ones16_col[:tcn], start=(ti == 0), stop=(ti == 6))
            nc.vector.tensor_copy(kscol[:, mc], psk)
        invd = rwp.tile([1, 2, 392], f32r, tag="invd")
        for h in range(2):
            psD = psr.tile([1, 392], f32, tag="psrow")
            for mc in range(2):
                nc.tensor.matmul(psD, kscol[:, mc], qp[:, mc, HSL[h]],
                                 start=(mc == 0), stop=(mc == 1))
            if GELU_IDENTITY:
                dreg = rwp.tile([1, 392], f32r, tag="t1")
                nc.vector.tensor_scalar_add(dreg, psD, 1e-8)
                nc.vector.reciprocal(invd[:, h], dreg)
            else:
                dr = rwp.tile([1, 392], f32r, tag="t1")
                nc.scalar.activation(dr, psD, ACTF.Abs_reciprocal_sqrt,
                                     bias=eps8_row)
                nc.vector.tensor_tensor(invd[:, h], dr, dr, ALU.mult)
        kptvT = tkp.tile([128, 2, 512], f16, tag="kptvT")
        for mc in range(2):
            ps = psp.tile([128, 512], f32, tag="ps")
            for ti in range(7):
                tcn = 128 if ti < 6 else 16
                nc.tensor.matmul(ps, kptok[:tcn, ti, mc * 128:(mc + 1) * 128],
                                 vtok[:tcn, ti], start=(ti == 0), stop=(ti == 6))
            nc.scalar.activation(kptvT[:, mc], ps, ACTF.Copy)
        y = tkp.tile([128, 4, 784], f16, tag="kfm")
        for h in range(2):
            psb = psp.tile([128, 392], f32, tag="ps")
            nc.tensor.matmul(psb, ones32_row, invd[:, h], start=True, stop=True)
            invb = sqp.tile([128, 392], f32r, tag="sq")
            nc.scalar.activation(invb, psb, ACTF.Copy)
            for vc in range(4):
                psy = psp.tile([128, 392], f32, tag="ps")
                for mc in range(2):
                    nc.tensor.matmul(psy, kptvT[:, mc, vc * 128:(vc + 1) * 128],
                                     qp[:, mc, HSL[h]],
                                     start=(mc == 0), stop=(mc == 1))
                nc.vector.tensor_tensor(y[:, vc, HSL[h]], psy, invb, ALU.mult)

        # ---------------- proj + residual into z
        for m in range(4):
            for h in range(2):
                ps = psp.tile([128, 392], f32, tag="ps")
                for kc in range(4):
                    nc.tensor.matmul(ps, w_proj[:, kc, m * 128:(m + 1) * 128],
                                     y[:, kc, HSL[h]],
                                     start=(kc == 0), stop=(kc == 3))
                tmp = sqp.tile([128, 392], f32r, tag="sq")
                nc.scalar.activation(tmp, ps, ACTF.Identity, bias=b_proj[:, m:m+1])
                nc.vector.tensor_add(z[:, m, HSL[h]], z[:, m, HSL[h]], tmp)

        # ---------------- LN2 + mlp
        zhat2 = tkp.tile([128, 4, 784], f16, tag="zhat")
        layer_norm(z, zhat2)
        g1t = tkp.tile([128, 4, 784], f16, tag="z1")
        for m in range(4):
            for h in range(2):
                ps = psp.tile([128, 392], f32, tag="ps")
                for kc in range(4):
                    nc.tensor.matmul(ps, w_mlp1[:, kc, m * 128:(m + 1) * 128],
                                     zhat2[:, kc, HSL[h]],
                                     start=(kc == 0), stop=(kc == 3))
                nc.scalar.activation(g1t[:, m, HSL[h]], ps, GELU,
                                     bias=b_mlp1[:, m:m+1])
        # mlp2 token-major + identity residual; write output
        for ti in range(7):
            tcn = 128 if ti < 6 else 16
            tsl = slice(ti * 128, ti * 128 + tcn)
            ps = psp.tile([128, 512], f32, tag="ps")
            for kc in range(4):
                nc.tensor.matmul(ps[:tcn], g1t[:, kc, tsl], w_mlp2[:, kc],
                                 start=(kc == 0), stop=False)
            for kc in range(4):
                nc.tensor.matmul(ps[:tcn], z[:, kc, tsl], w_i512[:, kc],
                                 start=False, stop=(kc == 3))
            osb = otp.tile([128, 512], f32, tag="osb")
            nc.vector.tensor_add(osb[:tcn], ps[:tcn], w_b2bc[:tcn])
            dma(d["d_out"][it, ti * 128: ti * 128 + tcn, :], osb[:tcn])


# ---------------------------------------------------------------- entry
_BUILD_CACHE = {}


def _get_nc(n_items):
    if n_items not in _BUILD_CACHE:
        _BUILD_CACHE[n_items] = _build_bass(n_items)
    return _BUILD_CACHE[n_items]


def kernel(a, b, c, params, w):
    from concourse.bass_utils import run_bass_kernel_spmd

    arrs = _prep_arrays(a, b, c, params, w)
    nc = _get_nc(ITEMS)

    shared = {k: v for k, v in arrs.items() if k not in ("a_t", "b_in", "c_in")}
    in_maps = []
    for core in range(N_CORES):
        s = slice(core * ITEMS, (core + 1) * ITEMS)
        m = dict(shared)
        m["a_t"] = arrs["a_t"][s]
        m["b_in"] = arrs["b_in"][s]
        m["c_in"] = arrs["c_in"][s]
        in_maps.append(m)

    res = run_bass_kernel_spmd(nc, in_maps, core_ids=list(range(N_CORES)))
    global LAST_RESULT
    LAST_RESULT = res
    out = np.concatenate([r["out"] for r in res.results], axis=0)
    return out.astype(np.float32)


LAST_RESULT = None


# revision 48
# speedup vs baseline: 1.1202x; 1.0790x over previous
"""Trainium2 Bass kernel for nn_FEM_33251636806316 (sparse_attention).

Data-parallel over batch: 64 items -> 8 NeuronCores x 8 items each.
All heavy matmuls in fp16 (full PE rate), psum fp32, residual stream and
LN statistics in float32r (fp32 storage, full-rate matmul for N>=256).
"""

import math

import numpy as np

# ---------------------------------------------------------------- constants
B = 64
HW = 14
EMB = 512
CUR = 1024
CH = 256  # conv channels
M = 256  # FAVOR+ features
T = 4 * HW * HW  # 784 tokens
N_CORES = 8
ITEMS = B // N_CORES  # 8 per core
NEG_LN16 = -math.log(math.sqrt(M))  # -ln(16)

# conv layer chunk table: (chunk_base, n_in_chunks) in the packed conv weight
_CONV_CHUNKS = [(0, 4), (4, 4), (8, 4), (12, 4), (16, 2)]

# debug: replace Gelu by Identity (CoreSim has no Gelu model)
GELU_IDENTITY = False


# ---------------------------------------------------------------- host prep
def _prep_arrays(a, b, c, params, w):
    """Host-side marshalling: weight folding, layout transforms, fp16 casts."""
    p = params
    f16 = np.float16
    f32 = np.float32

    def to16(x):
        return np.ascontiguousarray(np.asarray(x), dtype=f16)

    def to32(x):
        return np.ascontiguousarray(np.asarray(x), dtype=f32)

    arrs = {}
    # inputs
    arrs["a_t"] = to16(np.asarray(a).transpose(0, 2, 1))  # [B, 512, 784]
    arrs["b_in"] = to32(b)  # [B, 196, 1024]
    arrs["c_in"] = to32(np.asarray(c).reshape(B, CH, 28 * 28))  # [B, 256, 784]

    # conv weights: fold BN scale into W, transpose to [ic, tap, oc], chunked
    chunks = []
    bias = np.zeros((5, CH), f32)
    for i in range(1, 6):
        wi = np.asarray(p[f"cbr{i}_w"], f32)  # [oc, ic, 3, 3]
        si = np.asarray(p[f"cbr{i}_s"], f32)
        bi = np.asarray(p[f"cbr{i}_b"], f32)
        wi = wi * si[:, None, None, None]
        bias[i - 1] = bi
        wt = wi.transpose(1, 2, 3, 0).reshape(wi.shape[1], 9, CH)  # [ic,9,oc]
        nch = wi.shape[1] // 128
        chunks.append(wt.reshape(nch, 128, 9, CH))
    arrs["convw"] = to16(np.concatenate(chunks, 0))  # [18, 128, 9, 256]
    arrs["convb"] = bias  # [5, 256]

    arrs["cf1w"] = to16(p["cf_w1"])  # [768, 512]
    arrs["cf1b"] = to32(p["cf_b1"])
    arrs["cf2w"] = to16(p["cf_w2"])
    arrs["cf2b"] = to32(p["cf_b2"])

    # LN1 folded into kqv
    g1 = np.asarray(p["ln1_g"], f32)
    b1 = np.asarray(p["ln1_b"], f32)
    kqvw = np.asarray(p["kqv_w"], f32)
    kqvb = np.asarray(p["kqv_b"], f32)
    arrs["kqvw"] = to16(kqvw * g1[:, None])  # [512, 1536]
    c2 = b1 @ kqvw + kqvb  # [1536]
    arrs["kqvc2"] = to32(c2[:1024])  # k,q biases
    arrs["c2vbc"] = to16(np.tile(c2[1024:1536][None, :], (128, 1)))  # [128,512]

    arrs["wT"] = to16(np.asarray(w, f32).T)  # [512, 256]
    arrs["projw"] = to16(p["proj_w"])
    arrs["projb"] = to32(p["proj_b"])

    # LN2 folded into mlp1
    g2 = np.asarray(p["ln2_g"], f32)
    b2 = np.asarray(p["ln2_b"], f32)
    m1w = np.asarray(p["mlp_w1"], f32)
    arrs["mlp1w"] = to16(m1w * g2[:, None])
    arrs["mlp1c2"] = to32(b2 @ m1w + np.asarray(p["mlp_b1"], f32))
    arrs["mlp2w"] = to16(p["mlp_w2"])
    arrs["b2bc"] = to32(np.tile(np.asarray(p["mlp_b2"], f32)[None, :], (128, 1)))

    arrs["caw1"] = to16(p["ca_w1"])  # [256, 16]
    arrs["caw2"] = to16(p["ca_w2"])  # [16, 256]

    # spatial-attention banded matrices: band[ch*7+kx][y_in, y_out]
    saw = np.asarray(p["sa_w"], f32)[0]  # [2, 7, 7]
    bands = np.zeros((14, 28, 28), f32)
    for ch in range(2):
        for kx in range(7):
            for yo in range(28):
                for yi in range(max(0, yo - 3), min(28, yo + 4)):
                    bands[ch * 7 + kx, yi, yo] = saw[ch, yi - yo + 3, kx]
    bands[0:7] *= 1.0 / CH  # fold the channel-mean 1/256 into ch-0 bands
    arrs["bands"] = to16(bands)
    return arrs


# ---------------------------------------------------------------- bass build
def _build_bass(n_items):
    from contextlib import ExitStack

    import concourse.bass as bass
    import concourse.tile as tile
    from concourse import bacc, mybir
    from concourse.masks import make_identity

    f16 = mybir.dt.float16
    f32 = mybir.dt.float32
    f32r = mybir.dt.float32r
    AX = mybir.AxisListType
    ALU = mybir.AluOpType
    ACTF = mybir.ActivationFunctionType

    nc = bacc.Bacc("TRN2", target_bir_lowering=False, debug=False)

    # ---- dram tensors
    d_a = nc.dram_tensor("a_t", [n_items, EMB, T], f16, kind="ExternalInput").ap()
    d_b = nc.dram_tensor("b_in", [n_items, 196, CUR], f32, kind="ExternalInput").ap()
    d_c = nc.dram_tensor("c_in", [n_items, CH, 784], f32, kind="ExternalInput").ap()
    d_convw = nc.dram_tensor("convw", [18, 128, 9, CH], f16, kind="ExternalInput").ap()
    d_convb = nc.dram_tensor("convb", [5, CH], f32, kind="ExternalInput").ap()
    d_cf1w = nc.dram_tensor("cf1w", [768, 512], f16, kind="ExternalInput").ap()
    d_cf1b = nc.dram_tensor("cf1b", [512], f32, kind="ExternalInput").ap()
    d_cf2w = nc.dram_tensor("cf2w", [512, 512], f16, kind="ExternalInput").ap()
    d_cf2b = nc.dram_tensor("cf2b", [512], f32, kind="ExternalInput").ap()
    d_kqvw = nc.dram_tensor("kqvw", [512, 1536], f16, kind="ExternalInput").ap()
    d_kqvc2 = nc.dram_tensor("kqvc2", [1024], f32, kind="ExternalInput").ap()
    d_c2vbc = nc.dram_tensor("c2vbc", [128, 512], f16, kind="ExternalInput").ap()
    d_wT = nc.dram_tensor("wT", [512, 256], f16, kind="ExternalInput").ap()
    d_projw = nc.dram_tensor("projw", [512, 512], f16, kind="ExternalInput").ap()
    d_projb = nc.dram_tensor("projb", [512], f32, kind="ExternalInput").ap()
    d_mlp1w = nc.dram_tensor("mlp1w", [512, 512], f16, kind="ExternalInput").ap()
    d_mlp1c2 = nc.dram_tensor("mlp1c2", [512], f32, kind="ExternalInput").ap()
    d_mlp2w = nc.dram_tensor("mlp2w", [512, 512], f16, kind="ExternalInput").ap()
    d_b2bc = nc.dram_tensor("b2bc", [128, 512], f32, kind="ExternalInput").ap()
    d_caw1 = nc.dram_tensor("caw1", [CH, 16], f16, kind="ExternalInput").ap()
    d_caw2 = nc.dram_tensor("caw2", [16, CH], f16, kind="ExternalInput").ap()
    d_bands = nc.dram_tensor("bands", [14, 28, 28], f16, kind="ExternalInput").ap()
    d_out = nc.dram_tensor("out", [n_items, EMB, T], f32, kind="ExternalOutput").ap()

    with tile.TileContext(nc) as tc:
        with ExitStack() as ctx:
            with nc.allow_low_precision(reason="intentional fp16 matmul pipeline"):
                _emit(tc, ctx, nc, n_items, locals())
    nc.compile()
    return nc


def _emit(tc, ctx, nc, n_items, d):
    import concourse.bass as bass
    import concourse.bass_isa as bass_isa
    from concourse import mybir
    from concourse.masks import make_identity

    f16 = mybir.dt.float16
    f32 = mybir.dt.float32
    f32r = mybir.dt.float32r
    AX = mybir.AxisListType
    ALU = mybir.AluOpType
    ACTF = mybir.ActivationFunctionType
    GELU = ACTF.Identity if GELU_IDENTITY else ACTF.Gelu

    pool = lambda name, bufs: ctx.enter_context(tc.tile_pool(name=name, bufs=bufs))
    wp = pool("weights", 1)  # resident weights
    cwp = pool("convw", 3)  # streamed conv weights
    cvp = pool("convact", 1)  # conv activations
    stg = pool("stage", 1)  # input staging
    tkp = pool("tok", 1)  # token activations
    sqp = pool("sq", 2)  # small square/tmp tiles
    rwp = pool("rows", 1)  # [1, x] rows
    smp = pool("small", 1)  # small per-item tensors
    otp = pool("outsb", 2)  # output staging
    psp = ctx.enter_context(tc.tile_pool(name="psum", bufs=3, space="PSUM"))
    psr = ctx.enter_context(tc.tile_pool(name="psrow", bufs=2, space="PSUM"))

    dma = nc.sync.dma_start

    # ---------------- resident weights
    w_cf1 = wp.tile([128, 6, 512], f16, tag="w_cf1")
    dma(w_cf1, d["d_cf1w"].rearrange("(c p) n -> p c n", p=128))
    w_cf2 = wp.tile([128, 4, 512], f16, tag="w_cf2")
    dma(w_cf2, d["d_cf2w"].rearrange("(c p) n -> p c n", p=128))
    w_kqv = wp.tile([128, 4, 1536], f16, tag="w_kqv")
    dma(w_kqv, d["d_kqvw"].rearrange("(c p) n -> p c n", p=128))
    w_wT = wp.tile([128, 4, 256], f16, tag="w_wT")
    dma(w_wT, d["d_wT"].rearrange("(c p) n -> p c n", p=128))
    w_proj = wp.tile([128, 4, 512], f16, tag="w_proj")
    dma(w_proj, d["d_projw"].rearrange("(c p) n -> p c n", p=128))
    w_mlp1 = wp.tile([128, 4, 512], f16, tag="w_mlp1")
    dma(w_mlp1, d["d_mlp1w"].rearrange("(c p) n -> p c n", p=128))
    w_mlp2 = wp.tile([128, 4, 512], f16, tag="w_mlp2")
    dma(w_mlp2, d["d_mlp2w"].rearrange("(c p) n -> p c n", p=128))
    w_c2vbc = wp.tile([128, 512], f16, tag="w_c2vbc")
    dma(w_c2vbc, d["d_c2vbc"])
    b_mlp2 = wp.tile([128, 4], f32, tag="b_mlp2")
    dma(b_mlp2, d["d_b2bc"][0:1, :].rearrange("o (m p) -> (o p) m", p=128))
    w_caw1 = wp.tile([128, 2, 16], f16, tag="w_caw1")
    dma(w_caw1, d["d_caw1"].rearrange("(c p) n -> p c n", p=128))
    w_caw2 = wp.tile([16, 256], f16, tag="w_caw2")
    dma(w_caw2, d["d_caw2"])
    w_bands = wp.tile([28, 14, 28], f16, tag="w_bands")
    dma(w_bands, d["d_bands"].rearrange("i p j -> p i j"))
    b_conv = wp.tile([128, 5, 2], f32, tag="b_conv")
    dma(b_conv, d["d_convb"].rearrange("l (m p) -> p l m", p=128))
    b_cf1 = wp.tile([128, 4], f32, tag="b_cf1")
    dma(b_cf1, d["d_cf1b"].rearrange("(m p) -> p m", p=128))
    b_cf2 = wp.tile([128, 4], f32, tag="b_cf2")
    dma(b_cf2, d["d_cf2b"].rearrange("(m p) -> p m", p=128))
    b_kqv = wp.tile([128, 8], f32, tag="b_kqv")
    dma(b_kqv, d["d_kqvc2"].rearrange("(m p) -> p m", p=128))
    b_proj = wp.tile([128, 4], f32, tag="b_proj")
    dma(b_proj, d["d_projb"].rearrange("(m p) -> p m", p=128))
    b_mlp1 = wp.tile([128, 4], f32, tag="b_mlp1")
    dma(b_mlp1, d["d_mlp1c2"].rearrange("(m p) -> p m", p=128))

    ident = wp.tile([128, 128], f16, tag="ident")
    make_identity(nc, ident)
    ones16_col = wp.tile([128, 1], f16, tag="ones16_col")
    nc.gpsimd.memset(ones16_col, 1.0)
    neghalf_col = wp.tile([128, 1], f16, tag="neghalf_col")
    nc.gpsimd.memset(neghalf_col, -0.5)
    ones16_row = wp.tile([1, 256], f16, tag="ones16_row")
    nc.gpsimd.memset(ones16_row, 1.0)
    ones32_col_f = wp.tile([128, 1], f32, tag="ones32_col")
    nc.gpsimd.memset(ones32_col_f, 1.0)
    ones32_col = ones32_col_f.bitcast(f32r)
    ones32_row_f = wp.tile([1, 128], f32, tag="ones32_row")
    nc.gpsimd.memset(ones32_row_f, 1.0)
    ones32_row = ones32_row_f.bitcast(f32r)
    eps_row = wp.tile([1, 1], f32, tag="eps_row")
    nc.gpsimd.memset(eps_row, 1e-5)
    eps8_row = wp.tile([1, 1], f32, tag="eps8_row")
    nc.gpsimd.memset(eps8_row, 1e-8)
    # persistent padded conv tiles: borders zeroed once, interiors rewritten
    t_pad = wp.tile([128, 2, 30, 30], f16, tag="t_pad")
    nc.gpsimd.memset(t_pad, 0.0)
    c_pad = wp.tile([128, 2, 30, 30], f16, tag="c_pad")
    nc.gpsimd.memset(c_pad, 0.0)
    tx_pad = wp.tile([128, 2, 30, 30], f16, tag="tx_pad")
    nc.gpsimd.memset(tx_pad, 0.0)
    cx_pad = wp.tile([128, 2, 30, 30], f16, tag="cx_pad")
    nc.gpsimd.memset(cx_pad, 0.0)
    x1_pad = wp.tile([128, 2, 30, 30], f16, tag="x1_pad")
    nc.gpsimd.memset(x1_pad, 0.0)
    sa_t = wp.tile([28, 2, 34], f16, tag="sa_t")
    nc.gpsimd.memset(sa_t, 0.0)
    negln16_col = wp.tile([128, 1], f32, tag="negln16_col")
    nc.gpsimd.memset(negln16_col, NEG_LN16)

    HSL = [slice(0, 392), slice(392, 784)]  # token halves

    # =============================================================== items
    for it in range(n_items):
        # ---------------- input loads
        tb0 = stg.tile([128, 1024], f32, tag="bstage")
        dma(tb0, d["d_b"][it, 0:128, :])
        tc0 = stg.tile([128, 2, 784], f32, tag="c0")
        dma(tc0, d["d_c"][it].rearrange("(cj p) x -> p cj x", p=128))
        aT = stg.tile([128, 4, 784], f16, tag="aT")
        dma(aT, d["d_a"][it].rearrange("(cj p) t -> p cj t", p=128))

        # ---------------- pixel shuffle b -> t_pad
        b16_0 = stg.tile([128, 1024], f16, tag="b16_0")
        nc.scalar.activation(b16_0, tb0, ACTF.Copy)
        tb1 = stg.tile([128, 1024], f32, tag="bstage")
        dma(tb1[:68], d["d_b"][it, 128:196, :])
        b16_1 = stg.tile([68, 1024], f16, tag="b16_1")
        nc.scalar.activation(b16_1, tb1[:68], ACTF.Copy)
        bv0 = b16_0.rearrange("p (c g) -> p g c", g=4)
        bv1 = b16_1.rearrange("p (c g) -> p g c", g=4)
        for cj in range(2):
            for g in range(4):
                d1, d2 = g // 2, g % 2
                ps = psp.tile([128, 196], f16, tag="ps16")
                nc.tensor.transpose(
                    ps[:, 0:128], bv0[:, g, cj * 128:(cj + 1) * 128], ident)
                nc.tensor.transpose(
                    ps[:, 128:196], bv1[:, g, cj * 128:(cj + 1) * 128],
                    ident[:68, :68])
                dst = t_pad[:, cj, 1:29, 1:29].rearrange(
                    "p (i a) (j b) -> p a b i j", a=2, b=2)[:, d1, d2]
                nc.scalar.activation(dst, ps.rearrange("p (i j) -> p i j", j=14),
                                     ACTF.Copy)

        # ---------------- c load/pad + channel attention
        for cj in range(2):
            nc.scalar.activation(
                c_pad[:, cj, 1:29, 1:29],
                tc0[:, cj].rearrange("p (x y) -> p x y", y=28), ACTF.Copy)
        cred = smp.tile([128, 2, 2], f32, tag="cred")
        for cj in range(2):
            nc.vector.reduce_sum(cred[:, cj, 0:1], tc0[:, cj], axis=AX.X)
            nc.vector.reduce_max(cred[:, cj, 1:2], tc0[:, cj], axis=AX.X)
        v2 = smp.tile([128, 2, 2], f16, tag="v2")
        for cj in range(2):
            nc.vector.tensor_scalar_mul(v2[:, cj, 0:1], cred[:, cj, 0:1], 1.0 / 784)
            nc.vector.tensor_copy(v2[:, cj, 1:2], cred[:, cj, 1:2])
        psh = psp.tile([16, 2], f32, tag="ps")
        nc.tensor.matmul(psh, w_caw1[:, 0], v2[:, 0], start=True, stop=False)
        nc.tensor.matmul(psh, w_caw1[:, 1], v2[:, 1], start=False, stop=True)
        h16 = smp.tile([16, 2], f16, tag="h16")
        nc.scalar.activation(h16, psh, ACTF.Relu)
        cc = smp.tile([128, 2, 1], f32, tag="cc")
        for cj in range(2):
            ps2 = psp.tile([128, 1], f32, tag="ps")
            nc.tensor.matmul(ps2, w_caw2[:, cj * 128:(cj + 1) * 128], h16[:, 0:1],
                             start=True, stop=False)
            nc.tensor.matmul(ps2, w_caw2[:, cj * 128:(cj + 1) * 128], h16[:, 1:2],
                             start=False, stop=True)
            cct = smp.tile([128, 1], f32, tag="cct")
            nc.scalar.activation(cct, ps2, ACTF.Tanh, scale=0.5)
            nc.vector.tensor_scalar(cc[:, cj], cct, 0.5, 0.5,
                                    op0=ALU.mult, op1=ALU.add)

        # ---------------- spatial attention
        tadd = smp.tile([128, 28, 28], f16, tag="tadd")
        nc.vector.tensor_add(
            tadd, t_pad[:, 0, 1:29, 1:29], t_pad[:, 1, 1:29, 1:29])
        tmax = smp.tile([128, 28, 28], f16, tag="tmax")
        nc.vector.tensor_tensor(
            tmax, t_pad[:, 0, 1:29, 1:29], t_pad[:, 1, 1:29, 1:29], ALU.max)
        maxall = smp.tile([128, 28, 28], f16, tag="maxall")
        nc.gpsimd.partition_all_reduce(maxall, tmax, 128, bass_isa.ReduceOp.max)
        maxrow = maxall[0:1]
        meanrow = rwp.tile([1, 2, 392], f16, tag="meanrow")
        for h in range(2):
            psm = psr.tile([1, 14, 28], f32, tag="psrow")
            nc.tensor.matmul(
                psm, ones16_col, tadd[:, h * 14:(h + 1) * 14, :],
                start=True, stop=True)
            nc.scalar.activation(meanrow[:, h], psm.rearrange("o i j -> o (i j)"),
                                 ACTF.Copy)
        dma(sa_t[:, 0, 3:31], meanrow.rearrange("o h x -> o (h x)"))
        dma(sa_t[:, 1, 3:31], maxrow.rearrange("o x y -> o (x y)"))
        pssa = psp.tile([28, 28], f32, tag="ps")
        for ch in range(2):
            for kx in range(7):
                nc.tensor.matmul(
                    pssa, w_bands[:, ch * 7 + kx, :], sa_t[:28, ch, kx:kx + 28],
                    start=(ch == 0 and kx == 0), stop=(ch == 1 and kx == 6))
        ts28 = smp.tile([28, 28], f16, tag="ts28")
        tst = smp.tile([28, 28], f32, tag="tst")
        nc.scalar.activation(tst, pssa, ACTF.Tanh, scale=0.5)
        nc.vector.tensor_scalar(ts28, tst, 0.5, 0.5, op0=ALU.mult, op1=ALU.add)
        tsrow = rwp.tile([1, 784], f16, tag="tsrow")
        dma(tsrow, ts28.rearrange("p x -> p x"))

        # ---------------- t_x = t * cc, c_x = c * ts
        for cj in range(2):
            nc.vector.tensor_scalar_mul(tx_pad[:, cj], t_pad[:, cj], cc[:, cj])
        for h in range(2):
            psts = psp.tile([128, 14, 28], f32, tag="ps")
            nc.tensor.matmul(
                psts, ones16_row[:, 0:128],
                tsrow[:, h * 392:(h + 1) * 392].rearrange("o (i j) -> o i j", j=28),
                start=True, stop=True)
            for cj in range(2):
                r0 = 1 + h * 14
                nc.vector.tensor_tensor(
                    cx_pad[:, cj, r0:r0 + 14, 1:29],
                    c_pad[:, cj, r0:r0 + 14, 1:29], psts, ALU.mult)

        # ---------------- conv blocks
        def conv_block(lidx, in_specs, out_write):
            cb, nch = _CONV_CHUNKS[lidx]
            wt = []
            for half in range((nch + 1) // 2):
                cw = cwp.tile([128, 2, 9, CH], f16, tag="cw")
                n = min(2, nch - half * 2)
                dma(cw[:, 0:n],
                    d["d_convw"][cb + half * 2: cb + half * 2 + n].rearrange(
                        "c p t o -> p c t o"))
                wt.append(cw)
            for m in range(2):
                for h in range(2):
                    ps = psp.tile([128, 14, 28], f32, tag="ps")
                    idx = 0
                    last = nch * 9 - 1
                    for kc in range(nch):
                        tile_in, cj = in_specs[kc]
                        for dy in range(3):
                            for dx in range(3):
                                lhsT = wt[kc // 2][:, kc % 2, dy * 3 + dx,
                                                   m * 128:(m + 1) * 128]
                                rhs = tile_in[:, cj, h * 14 + dy: h * 14 + dy + 14,
                                              dx:dx + 28]
                                nc.tensor.matmul(ps, lhsT, rhs,
                                                 start=(idx == 0), stop=(idx == last))
                                idx += 1
                    out_write(m, h, ps)

        def pad_writer(dst, lidx):
            def write(m, h, ps):
                o = dst[:, m, 1 + h * 14: 15 + h * 14, 1:29]
                if h == 0:
                    nc.scalar.activation(o, ps, ACTF.Relu, bias=b_conv[:, lidx, m:m+1])
                else:
                    nc.vector.tensor_scalar(o, ps, b_conv[:, lidx, m:m+1], 0.0,
                                            op0=ALU.add, op1=ALU.max)
            return write

        conv_block(0, [(tx_pad, 0), (tx_pad, 1), (cx_pad, 0), (cx_pad, 1)],
                   pad_writer(x1_pad, 0))
        u_pad = tx_pad
        conv_block(1, [(t_pad, 0), (t_pad, 1), (x1_pad, 0), (x1_pad, 1)],
                   pad_writer(u_pad, 1))
        cx2_pad = t_pad
        conv_block(2, [(c_pad, 0), (c_pad, 1), (x1_pad, 0), (x1_pad, 1)],
                   pad_writer(cx2_pad, 2))
        x2_pad = cx_pad
        conv_block(3, [(u_pad, 0), (u_pad, 1), (cx2_pad, 0), (cx2_pad, 1)],
                   pad_writer(x2_pad, 3))
        bt = cvp.tile([128, 2, 28, 28], f16, tag="bt")

        def bt_writer(m, h, ps):
            o = bt[:, m, h * 14:(h + 1) * 14, :]
            if h == 0:
                nc.scalar.activation(o, ps, ACTF.Relu, bias=b_conv[:, 4, m:m+1])
            else:
                nc.vector.tensor_scalar(o, ps, b_conv[:, 4, m:m+1], 0.0,
                                        op0=ALU.add, op1=ALU.max)

        conv_block(4, [(x2_pad, 0), (x2_pad, 1)], bt_writer)
        btf = bt.rearrange("p m x y -> p m (x y)")

        # ---------------- cf mlp: z1 = gelu(cat @ W1 + b1); z = z1 @ W2 + b2
        z1 = tkp.tile([128, 4, 784], f16, tag="z1")
        for m in range(4):
            for h in range(2):
                ps = psp.tile([128, 392], f32, tag="ps")
                for kc in range(6):
                    rhs = (aT[:, kc, HSL[h]] if kc < 4
                           else btf[:, kc - 4, HSL[h]])
                    nc.tensor.matmul(ps, w_cf1[:, kc, m * 128:(m + 1) * 128], rhs,
                                     start=(kc == 0), stop=(kc == 5))
                nc.scalar.activation(z1[:, m, HSL[h]], ps, GELU,
                                     bias=b_cf1[:, m:m+1])
        z = tkp.tile([128, 4, 784], f32r, tag="z")
        for m in range(4):
            for h in range(2):
                ps = psp.tile([128, 392], f32, tag="ps")
                for kc in range(4):
                    nc.tensor.matmul(ps, w_cf2[:, kc, m * 128:(m + 1) * 128],
                                     z1[:, kc, HSL[h]],
                                     start=(kc == 0), stop=(kc == 3))
                nc.scalar.activation(z[:, m, HSL[h]], ps, ACTF.Identity,
                                     bias=b_cf2[:, m:m+1])

        # ---------------- layernorm helper (stats + normalize to fp16)
        def layer_norm(src, dst):
            rstd = rwp.tile([1, 2, 392], f32r, tag="rstd")
            m2 = rwp.tile([1, 2, 392], f32r, tag="m2")
            for h in range(2):
                psA = psr.tile([1, 392], f32, tag="psrow")
                psB = psr.tile([1, 392], f32, tag="psrow")
                for kc in range(4):
                    nc.tensor.matmul(psA, ones32_col, src[:, kc, HSL[h]],
                                     start=(kc == 0), stop=(kc == 3))
                for kc in range(4):
                    sqt = sqp.tile([128, 392], f32r, tag="sq")
                    nc.vector.tensor_tensor(sqt, src[:, kc, HSL[h]],
                                            src[:, kc, HSL[h]], ALU.mult)
                    nc.tensor.matmul(psB, ones32_col, sqt,
                                     start=(kc == 0), stop=(kc == 3))
                s1row = rwp.tile([1, 392], f32r, tag="s1row")
                nc.scalar.activation(s1row, psA, ACTF.Copy)
                t1 = rwp.tile([1, 392], f32r, tag="t1")
                nc.vector.tensor_tensor(t1, s1row, s1row, ALU.mult)
                nc.vector.tensor_scalar_mul(t1, t1, -1.0 / EMB)
                t2 = rwp.tile([1, 392], f32r, tag="t2")
                nc.vector.tensor_add(t2, psB, t1)
                if GELU_IDENTITY:  # CoreSim lacks Abs_reciprocal_sqrt
                    sdev = rwp.tile([1, 392], f32r, tag="t1")
                    nc.scalar.activation(sdev, t2, ACTF.Sqrt,
                                         bias=eps_row, scale=1.0 / EMB)
                    nc.vector.reciprocal(rstd[:, h], sdev)
                else:
                    nc.scalar.activation(rstd[:, h], t2, ACTF.Abs_reciprocal_sqrt,
                                         bias=eps_row, scale=1.0 / EMB)
                nc.vector.tensor_tensor(m2[:, h], s1row, rstd[:, h], ALU.mult)
                nc.vector.tensor_scalar_mul(m2[:, h], m2[:, h], 1.0 / EMB)
            for h in range(2):
                psrd = psp.tile([128, 392], f32, tag="ps")
                nc.tensor.matmul(psrd, ones32_row, rstd[:, h], start=True, stop=True)
                psm2 = psp.tile([128, 392], f32, tag="ps")
                nc.tensor.matmul(psm2, ones32_row, m2[:, h], start=True, stop=True)
                for kc in range(4):
                    tmpt = sqp.tile([128, 392], f32r, tag="sq")
                    nc.vector.tensor_tensor(tmpt, src[:, kc, HSL[h]], psrd, ALU.mult)
                    nc.vector.tensor_tensor(dst[:, kc, HSL[h]], tmpt, psm2,
                                            ALU.subtract)

        zhat = tkp.tile([128, 4, 784], f16, tag="zhat")
        layer_norm(z, zhat)

        # ---------------- kqv: k,q feature-major; v token-major
        kfm = tkp.tile([128, 4, 784], f16, tag="kfm")
        qfm = tkp.tile([128, 4, 784], f16, tag="qfm")
        for mi in range(8):
            dstT, mloc = (kfm, mi) if mi < 4 else (qfm, mi - 4)
            for h in range(2):
                ps = psp.tile([128, 392], f32, tag="ps")
                for kc in range(4):
                    nc.tensor.matmul(ps, w_kqv[:, kc, mi * 128:(mi + 1) * 128],
                                     zhat[:, kc, HSL[h]],
                                     start=(kc == 0), stop=(kc == 3))
                nc.scalar.activation(dstT[:, mloc, HSL[h]], ps, ACTF.Identity,
                                     bias=b_kqv[:, mi:mi+1])
        vtok = tkp.tile([128, 7, 512], f16, tag="vtok")
        for ti in range(7):
            tcn = 128 if ti < 6 else 16
            tsl = slice(ti * 128, ti * 128 + tcn)
            ps = psp.tile([128, 512], f32, tag="ps")
            for kc in range(4):
                nc.tensor.matmul(ps[:tcn], zhat[:, kc, tsl], w_kqv[:, kc, 1024:1536],
                                 start=(kc == 0), stop=(kc == 3))
            nc.vector.tensor_add(vtok[:tcn, ti], ps[:tcn], w_c2vbc[:tcn])

        # ---------------- FAVOR+ feature maps
        def xd_rows(src, tag):
            xd = rwp.tile([1, 2, 392], f16, tag=tag)
            for h in range(2):
                psx = psr.tile([1, 392], f32, tag="psrow")
                for kc in range(4):
                    sqt = sqp.tile([128, 392], f16, tag="sqh")
                    nc.vector.tensor_tensor(sqt, src[:, kc, HSL[h]],
                                            src[:, kc, HSL[h]], ALU.mult)
                    nc.tensor.matmul(psx, neghalf_col, sqt,
                                     start=(kc == 0), stop=(kc == 3))
                nc.scalar.activation(xd[:, h], psx, ACTF.Copy)
            return xd

        xdk = xd_rows(kfm, "xdk")
        xdq = xd_rows(qfm, "xdq")
        kptok = tkp.tile([128, 7, 256], f16, tag="kptok")
        xdk_flat = xdk.rearrange("o h x -> o (h x)")
        for ti in range(7):
            tcn = 128 if ti < 6 else 16
            tsl = slice(ti * 128, ti * 128 + tcn)
            ps = psp.tile([128, 256], f32, tag="ps")
            for kc in range(4):
                nc.tensor.matmul(ps[:tcn], kfm[:, kc, tsl], w_wT[:, kc],
                                 start=(kc == 0), stop=False)
            nc.tensor.matmul(ps[:tcn], xdk_flat[:, tsl], ones16_row,
                             start=False, stop=True)
            nc.scalar.activation(kptok[:tcn, ti], ps[:tcn], ACTF.Exp,
                                 bias=negln16_col[:tcn])
        qp = tkp.tile([128, 2, 784], f16, tag="qp")
        for mi in range(2):
            for h in range(2):
                ps = psp.tile([128, 392], f32, tag="ps")
                for kc in range(4):
                    nc.tensor.matmul(ps, w_wT[:, kc, mi * 128:(mi + 1) * 128],
                                     qfm[:, kc, HSL[h]],
                                     start=(kc == 0), stop=False)
                nc.tensor.matmul(ps, ones16_row[:, 0:128], xdq[:, h],
                                 start=False, stop=True)
                nc.scalar.activation(qp[:, mi, HSL[h]], ps, ACTF.Exp,
                                     bias=negln16_col)

        # ---------------- ksum, D, kptv, y
        kscol = smp.tile([128, 2, 1], f16, tag="kscol")
        for mc in range(2):
            psk = psp.tile([128, 1], f32, tag="ps")
            for ti in range(7):
                tcn = 128 if ti < 6 else 16
                nc.tensor.matmul(psk, kptok[:tcn, ti, mc * 128:(mc + 1) * 128],
                                 ones16_col[:tcn], start=(ti == 0), stop=(ti == 6))
            nc.vector.tensor_copy(kscol[:, mc], psk)
        invd = rwp.tile([1, 2, 392], f32r, tag="invd")
        for h in range(2):
            psD = psr.tile([1, 392], f32, tag="psrow")
            for mc in range(2):
                nc.tensor.matmul(psD, kscol[:, mc], qp[:, mc, HSL[h]],
                                 start=(mc == 0), stop=(mc == 1))
            if GELU_IDENTITY:
                dreg = rwp.tile([1, 392], f32r, tag="t1")
                nc.vector.tensor_scalar_add(dreg, psD, 1e-8)
                nc.vector.reciprocal(invd[:, h], dreg)
            else:
                dr = rwp.tile([1, 392], f32r, tag="t1")
                nc.scalar.activation(dr, psD, ACTF.Abs_reciprocal_sqrt,
                                     bias=eps8_row)
                nc.vector.tensor_tensor(invd[:, h], dr, dr, ALU.mult)
        kptvT = tkp.tile([128, 2, 512], f16, tag="kptvT")
        for mc in range(2):
            ps = psp.tile([128, 512], f32, tag="ps")
            for ti in range(7):
                tcn = 128 if ti < 6 else 16
                nc.tensor.matmul(ps, kptok[:tcn, ti, mc * 128:(mc + 1) * 128],
                                 vtok[:tcn, ti], start=(ti == 0), stop=(ti == 6))
            nc.scalar.activation(kptvT[:, mc], ps, ACTF.Copy)
        y = tkp.tile([128, 4, 784], f16, tag="kfm")
        for h in range(2):
            psb = psp.tile([128, 392], f32, tag="ps")
            nc.tensor.matmul(psb, ones32_row, invd[:, h], start=True, stop=True)
            invb = sqp.tile([128, 392], f32r, tag="sq")
            nc.scalar.activation(invb, psb, ACTF.Copy)
            for vc in range(4):
                psy = psp.tile([128, 392], f32, tag="ps")
                for mc in range(2):
                    nc.tensor.matmul(psy, kptvT[:, mc, vc * 128:(vc + 1) * 128],
                                     qp[:, mc, HSL[h]],
                                     start=(mc == 0), stop=(mc == 1))
                nc.vector.tensor_tensor(y[:, vc, HSL[h]], psy, invb, ALU.mult)

        # ---------------- proj + residual into z
        for m in range(4):
            for h in range(2):
                ps = psp.tile([128, 392], f32, tag="ps")
                for kc in range(4):
                    nc.tensor.matmul(ps, w_proj[:, kc, m * 128:(m + 1) * 128],
                                     y[:, kc, HSL[h]],
                                     start=(kc == 0), stop=(kc == 3))
                tmp = sqp.tile([128, 392], f32r, tag="sq")
                nc.scalar.activation(tmp, ps, ACTF.Identity, bias=b_proj[:, m:m+1])
                nc.vector.tensor_add(z[:, m, HSL[h]], z[:, m, HSL[h]], tmp)

        # ---------------- LN2 + mlp
        zhat2 = tkp.tile([128, 4, 784], f16, tag="zhat")
        layer_norm(z, zhat2)
        g1t = tkp.tile([128, 4, 784], f16, tag="z1")
        for m in range(4):
            for h in range(2):
                ps = psp.tile([128, 392], f32, tag="ps")
                for kc in range(4):
                    nc.tensor.matmul(ps, w_mlp1[:, kc, m * 128:(m + 1) * 128],
                                     zhat2[:, kc, HSL[h]],
                                     start=(kc == 0), stop=(kc == 3))
                nc.scalar.activation(g1t[:, m, HSL[h]], ps, GELU,
                                     bias=b_mlp1[:, m:m+1])
        # mlp2 feature-major + residual; output feature-major (host transposes)
        z3 = tkp.tile([128, 4, 784], f32r, tag="z3")
        for m in range(4):
            for h in range(2):
                ps = psp.tile([128, 392], f32, tag="ps")
                for kc in range(4):
                    nc.tensor.matmul(ps, w_mlp2[:, kc, m * 128:(m + 1) * 128],
                                     g1t[:, kc, HSL[h]],
                                     start=(kc == 0), stop=(kc == 3))
                tmp = sqp.tile([128, 392], f32r, tag="sq")
                nc.scalar.activation(tmp, ps, ACTF.Identity, bias=b_mlp2[:, m:m+1])
                nc.vector.tensor_add(z3[:, m, HSL[h]], z[:, m, HSL[h]], tmp)
        dma(d["d_out"][it].rearrange("(c p) t -> p c t", p=128), z3.bitcast(f32))


# ---------------------------------------------------------------- entry
_BUILD_CACHE = {}


def _get_nc(n_items):
    if n_items not in _BUILD_CACHE:
        _BUILD_CACHE[n_items] = _build_bass(n_items)
    return _BUILD_CACHE[n_items]


def kernel(a, b, c, params, w):
    from concourse.bass_utils import run_bass_kernel_spmd

    arrs = _prep_arrays(a, b, c, params, w)
    nc = _get_nc(ITEMS)

    shared = {k: v for k, v in arrs.items() if k not in ("a_t", "b_in", "c_in")}
    in_maps = []
    for core in range(N_CORES):
        s = slice(core * ITEMS, (core + 1) * ITEMS)
        m = dict(shared)
        m["a_t"] = arrs["a_t"][s]
        m["b_in"] = arrs["b_in"][s]
        m["c_in"] = arrs["c_in"][s]
        in_maps.append(m)

    res = run_bass_kernel_spmd(nc, in_maps, core_ids=list(range(N_CORES)))
    global LAST_RESULT
    LAST_RESULT = res
    out = np.concatenate([r["out"] for r in res.results], axis=0)
    return np.ascontiguousarray(out.transpose(0, 2, 1), dtype=np.float32)


LAST_RESULT = None


# revision 49
# speedup vs baseline: 1.1306x; 1.0093x over previous
"""Trainium2 Bass kernel for nn_FEM_33251636806316 (sparse_attention).

Data-parallel over batch: 64 items -> 8 NeuronCores x 8 items each.
All heavy matmuls in fp16 (full PE rate), psum fp32, residual stream and
LN statistics in float32r (fp32 storage, full-rate matmul for N>=256).
"""

import math

import numpy as np

# ---------------------------------------------------------------- constants
B = 64
HW = 14
EMB = 512
CUR = 1024
CH = 256  # conv channels
M = 256  # FAVOR+ features
T = 4 * HW * HW  # 784 tokens
N_CORES = 8
ITEMS = B // N_CORES  # 8 per core
NEG_LN16 = -math.log(math.sqrt(M))  # -ln(16)

# conv layer chunk table: (chunk_base, n_in_chunks) in the packed conv weight
_CONV_CHUNKS = [(0, 4), (4, 4), (8, 4), (12, 4), (16, 2)]

# debug: replace Gelu by Identity (CoreSim has no Gelu model)
GELU_IDENTITY = False


# ---------------------------------------------------------------- host prep
def _prep_arrays(a, b, c, params, w):
    """Host-side marshalling: weight folding, layout transforms, fp16 casts."""
    p = params
    f16 = np.float16
    f32 = np.float32

    def to16(x):
        return np.ascontiguousarray(np.asarray(x), dtype=f16)

    def to32(x):
        return np.ascontiguousarray(np.asarray(x), dtype=f32)

    arrs = {}
    # inputs
    arrs["a_t"] = to16(np.asarray(a).transpose(0, 2, 1))  # [B, 512, 784]
    arrs["b_in"] = to32(b)  # [B, 196, 1024]
    arrs["c_in"] = to32(np.asarray(c).reshape(B, CH, 28 * 28))  # [B, 256, 784]

    # conv weights: fold BN scale into W, transpose to [ic, tap, oc], chunked
    chunks = []
    bias = np.zeros((5, CH), f32)
    for i in range(1, 6):
        wi = np.asarray(p[f"cbr{i}_w"], f32)  # [oc, ic, 3, 3]
        si = np.asarray(p[f"cbr{i}_s"], f32)
        bi = np.asarray(p[f"cbr{i}_b"], f32)
        wi = wi * si[:, None, None, None]
        bias[i - 1] = bi
        wt = wi.transpose(1, 2, 3, 0).reshape(wi.shape[1], 9, CH)  # [ic,9,oc]
        nch = wi.shape[1] // 128
        chunks.append(wt.reshape(nch, 128, 9, CH))
    arrs["convw"] = to16(np.concatenate(chunks, 0))  # [18, 128, 9, 256]
    arrs["convb"] = bias  # [5, 256]

    arrs["cf1w"] = to16(p["cf_w1"])  # [768, 512]
    arrs["cf1b"] = to32(p["cf_b1"])
    arrs["cf2w"] = to16(p["cf_w2"])
    arrs["cf2b"] = to32(p["cf_b2"])

    # LN1 folded into kqv
    g1 = np.asarray(p["ln1_g"], f32)
    b1 = np.asarray(p["ln1_b"], f32)
    kqvw = np.asarray(p["kqv_w"], f32)
    kqvb = np.asarray(p["kqv_b"], f32)
    arrs["kqvw"] = to16(kqvw * g1[:, None])  # [512, 1536]
    c2 = b1 @ kqvw + kqvb  # [1536]
    arrs["kqvc2"] = to32(c2[:1024])  # k,q biases
    arrs["c2vbc"] = to16(np.tile(c2[1024:1536][None, :], (128, 1)))  # [128,512]

    arrs["wT"] = to16(np.asarray(w, f32).T)  # [512, 256]
    arrs["projw"] = to16(p["proj_w"])
    arrs["projb"] = to32(p["proj_b"])

    # LN2 folded into mlp1
    g2 = np.asarray(p["ln2_g"], f32)
    b2 = np.asarray(p["ln2_b"], f32)
    m1w = np.asarray(p["mlp_w1"], f32)
    arrs["mlp1w"] = to16(m1w * g2[:, None])
    arrs["mlp1c2"] = to32(b2 @ m1w + np.asarray(p["mlp_b1"], f32))
    arrs["mlp2w"] = to16(p["mlp_w2"])
    arrs["b2bc"] = to32(np.tile(np.asarray(p["mlp_b2"], f32)[None, :], (128, 1)))

    arrs["caw1"] = to16(p["ca_w1"])  # [256, 16]
    arrs["caw2"] = to16(p["ca_w2"])  # [16, 256]

    # spatial-attention banded matrices: band[ch*7+kx][y_in, y_out]
    saw = np.asarray(p["sa_w"], f32)[0]  # [2, 7, 7]
    bands = np.zeros((14, 28, 28), f32)
    for ch in range(2):
        for kx in range(7):
            for yo in range(28):
                for yi in range(max(0, yo - 3), min(28, yo + 4)):
                    bands[ch * 7 + kx, yi, yo] = saw[ch, yi - yo + 3, kx]
    bands[0:7] *= 1.0 / CH  # fold the channel-mean 1/256 into ch-0 bands
    arrs["bands"] = to16(bands)
    return arrs


# ---------------------------------------------------------------- bass build
def _build_bass(n_items):
    from contextlib import ExitStack

    import concourse.bass as bass
    import concourse.tile as tile
    from concourse import bacc, mybir
    from concourse.masks import make_identity

    f16 = mybir.dt.float16
    f32 = mybir.dt.float32
    f32r = mybir.dt.float32r
    AX = mybir.AxisListType
    ALU = mybir.AluOpType
    ACTF = mybir.ActivationFunctionType

    nc = bacc.Bacc("TRN2", target_bir_lowering=False, debug=False)

    # ---- dram tensors
    d_a = nc.dram_tensor("a_t", [n_items, EMB, T], f16, kind="ExternalInput").ap()
    d_b = nc.dram_tensor("b_in", [n_items, 196, CUR], f32, kind="ExternalInput").ap()
    d_c = nc.dram_tensor("c_in", [n_items, CH, 784], f32, kind="ExternalInput").ap()
    d_convw = nc.dram_tensor("convw", [18, 128, 9, CH], f16, kind="ExternalInput").ap()
    d_convb = nc.dram_tensor("convb", [5, CH], f32, kind="ExternalInput").ap()
    d_cf1w = nc.dram_tensor("cf1w", [768, 512], f16, kind="ExternalInput").ap()
    d_cf1b = nc.dram_tensor("cf1b", [512], f32, kind="ExternalInput").ap()
    d_cf2w = nc.dram_tensor("cf2w", [512, 512], f16, kind="ExternalInput").ap()
    d_cf2b = nc.dram_tensor("cf2b", [512], f32, kind="ExternalInput").ap()
    d_kqvw = nc.dram_tensor("kqvw", [512, 1536], f16, kind="ExternalInput").ap()
    d_kqvc2 = nc.dram_tensor("kqvc2", [1024], f32, kind="ExternalInput").ap()
    d_c2vbc = nc.dram_tensor("c2vbc", [128, 512], f16, kind="ExternalInput").ap()
    d_wT = nc.dram_tensor("wT", [512, 256], f16, kind="ExternalInput").ap()
    d_projw = nc.dram_tensor("projw", [512, 512], f16, kind="ExternalInput").ap()
    d_projb = nc.dram_tensor("projb", [512], f32, kind="ExternalInput").ap()
    d_mlp1w = nc.dram_tensor("mlp1w", [512, 512], f16, kind="ExternalInput").ap()
    d_mlp1c2 = nc.dram_tensor("mlp1c2", [512], f32, kind="ExternalInput").ap()
    d_mlp2w = nc.dram_tensor("mlp2w", [512, 512], f16, kind="ExternalInput").ap()
    d_b2bc = nc.dram_tensor("b2bc", [128, 512], f32, kind="ExternalInput").ap()
    d_caw1 = nc.dram_tensor("caw1", [CH, 16], f16, kind="ExternalInput").ap()
    d_caw2 = nc.dram_tensor("caw2", [16, CH], f16, kind="ExternalInput").ap()
    d_bands = nc.dram_tensor("bands", [14, 28, 28], f16, kind="ExternalInput").ap()
    d_out = nc.dram_tensor("out", [n_items, EMB, T], f32, kind="ExternalOutput").ap()

    with tile.TileContext(nc) as tc:
        with ExitStack() as ctx:
            with nc.allow_low_precision(reason="intentional fp16 matmul pipeline"):
                _emit(tc, ctx, nc, n_items, locals())
    nc.compile()
    return nc


def _emit(tc, ctx, nc, n_items, d):
    import concourse.bass as bass
    import concourse.bass_isa as bass_isa
    from concourse import mybir
    from concourse.masks import make_identity

    f16 = mybir.dt.float16
    f32 = mybir.dt.float32
    f32r = mybir.dt.float32r
    AX = mybir.AxisListType
    ALU = mybir.AluOpType
    ACTF = mybir.ActivationFunctionType
    GELU = ACTF.Identity if GELU_IDENTITY else ACTF.Gelu

    pool = lambda name, bufs: ctx.enter_context(tc.tile_pool(name=name, bufs=bufs))
    wp = pool("weights", 1)  # resident weights
    cwp = pool("convw", 3)  # streamed conv weights
    cvp = pool("convact", 1)  # conv activations
    stg = pool("stage", 1)  # input staging
    stb = pool("bstage", 2)  # b staging (double-buffered)
    tkp = pool("tok", 1)  # token activations
    sqp = pool("sq", 2)  # small square/tmp tiles
    rwp = pool("rows", 1)  # [1, x] rows
    smp = pool("small", 1)  # small per-item tensors
    otp = pool("outsb", 2)  # output staging
    psp = ctx.enter_context(tc.tile_pool(name="psum", bufs=4, space="PSUM"))
    pst = ctx.enter_context(tc.tile_pool(name="psumT", bufs=2, space="PSUM"))
    psr = ctx.enter_context(tc.tile_pool(name="psrow", bufs=2, space="PSUM"))

    dma = nc.sync.dma_start

    # ---------------- resident weights
    w_cf1 = wp.tile([128, 6, 512], f16, tag="w_cf1")
    dma(w_cf1, d["d_cf1w"].rearrange("(c p) n -> p c n", p=128))
    w_cf2 = wp.tile([128, 4, 512], f16, tag="w_cf2")
    dma(w_cf2, d["d_cf2w"].rearrange("(c p) n -> p c n", p=128))
    w_kqv = wp.tile([128, 4, 1536], f16, tag="w_kqv")
    dma(w_kqv, d["d_kqvw"].rearrange("(c p) n -> p c n", p=128))
    w_wT = wp.tile([128, 4, 256], f16, tag="w_wT")
    dma(w_wT, d["d_wT"].rearrange("(c p) n -> p c n", p=128))
    w_proj = wp.tile([128, 4, 512], f16, tag="w_proj")
    dma(w_proj, d["d_projw"].rearrange("(c p) n -> p c n", p=128))
    w_mlp1 = wp.tile([128, 4, 512], f16, tag="w_mlp1")
    dma(w_mlp1, d["d_mlp1w"].rearrange("(c p) n -> p c n", p=128))
    w_mlp2 = wp.tile([128, 4, 512], f16, tag="w_mlp2")
    dma(w_mlp2, d["d_mlp2w"].rearrange("(c p) n -> p c n", p=128))
    w_c2vbc = wp.tile([128, 512], f16, tag="w_c2vbc")
    dma(w_c2vbc, d["d_c2vbc"])
    b_mlp2 = wp.tile([128, 4], f32, tag="b_mlp2")
    dma(b_mlp2, d["d_b2bc"][0:1, :].rearrange("o (m p) -> (o p) m", p=128))
    w_caw1 = wp.tile([128, 2, 16], f16, tag="w_caw1")
    dma(w_caw1, d["d_caw1"].rearrange("(c p) n -> p c n", p=128))
    w_caw2 = wp.tile([16, 256], f16, tag="w_caw2")
    dma(w_caw2, d["d_caw2"])
    w_bands = wp.tile([28, 14, 28], f16, tag="w_bands")
    dma(w_bands, d["d_bands"].rearrange("i p j -> p i j"))
    b_conv = wp.tile([128, 5, 2], f32, tag="b_conv")
    dma(b_conv, d["d_convb"].rearrange("l (m p) -> p l m", p=128))
    b_cf1 = wp.tile([128, 4], f32, tag="b_cf1")
    dma(b_cf1, d["d_cf1b"].rearrange("(m p) -> p m", p=128))
    b_cf2 = wp.tile([128, 4], f32, tag="b_cf2")
    dma(b_cf2, d["d_cf2b"].rearrange("(m p) -> p m", p=128))
    b_kqv = wp.tile([128, 8], f32, tag="b_kqv")
    dma(b_kqv, d["d_kqvc2"].rearrange("(m p) -> p m", p=128))
    b_proj = wp.tile([128, 4], f32, tag="b_proj")
    dma(b_proj, d["d_projb"].rearrange("(m p) -> p m", p=128))
    b_mlp1 = wp.tile([128, 4], f32, tag="b_mlp1")
    dma(b_mlp1, d["d_mlp1c2"].rearrange("(m p) -> p m", p=128))

    ident = wp.tile([128, 128], f16, tag="ident")
    make_identity(nc, ident)
    ones16_col = wp.tile([128, 1], f16, tag="ones16_col")
    nc.gpsimd.memset(ones16_col, 1.0)
    neghalf_col = wp.tile([128, 1], f16, tag="neghalf_col")
    nc.gpsimd.memset(neghalf_col, -0.5)
    ones16_row = wp.tile([1, 256], f16, tag="ones16_row")
    nc.gpsimd.memset(ones16_row, 1.0)
    ones32_col_f = wp.tile([128, 1], f32, tag="ones32_col")
    nc.gpsimd.memset(ones32_col_f, 1.0)
    ones32_col = ones32_col_f.bitcast(f32r)
    ones32_row_f = wp.tile([1, 128], f32, tag="ones32_row")
    nc.gpsimd.memset(ones32_row_f, 1.0)
    ones32_row = ones32_row_f.bitcast(f32r)
    eps_row = wp.tile([1, 1], f32, tag="eps_row")
    nc.gpsimd.memset(eps_row, 1e-5)
    eps8_row = wp.tile([1, 1], f32, tag="eps8_row")
    nc.gpsimd.memset(eps8_row, 1e-8)
    # persistent padded conv tiles: borders zeroed once, interiors rewritten
    t_pad = wp.tile([128, 2, 30, 30], f16, tag="t_pad")
    nc.gpsimd.memset(t_pad, 0.0)
    c_pad = wp.tile([128, 2, 30, 30], f16, tag="c_pad")
    nc.gpsimd.memset(c_pad, 0.0)
    tx_pad = wp.tile([128, 2, 30, 30], f16, tag="tx_pad")
    nc.gpsimd.memset(tx_pad, 0.0)
    cx_pad = wp.tile([128, 2, 30, 30], f16, tag="cx_pad")
    nc.gpsimd.memset(cx_pad, 0.0)
    x1_pad = wp.tile([128, 2, 30, 30], f16, tag="x1_pad")
    nc.gpsimd.memset(x1_pad, 0.0)
    sa_t = wp.tile([28, 2, 34], f16, tag="sa_t")
    nc.gpsimd.memset(sa_t, 0.0)
    negln16_col = wp.tile([128, 1], f32, tag="negln16_col")
    nc.gpsimd.memset(negln16_col, NEG_LN16)

    HSL = [slice(0, 392), slice(392, 784)]  # token halves

    # =============================================================== items
    for it in range(n_items):
        # ---------------- input loads
        tb0 = stb.tile([128, 1024], f32, tag="bstage")
        dma(tb0, d["d_b"][it, 0:128, :])
        tc0 = stg.tile([128, 2, 784], f32, tag="c0")
        dma(tc0, d["d_c"][it].rearrange("(cj p) x -> p cj x", p=128))
        aT = stg.tile([128, 4, 784], f16, tag="aT")
        dma(aT, d["d_a"][it].rearrange("(cj p) t -> p cj t", p=128))

        # ---------------- pixel shuffle b -> t_pad
        b16_0 = stg.tile([128, 1024], f16, tag="b16_0")
        nc.scalar.activation(b16_0, tb0, ACTF.Copy)
        tb1 = stb.tile([128, 1024], f32, tag="bstage")
        dma(tb1[:68], d["d_b"][it, 128:196, :])
        b16_1 = stg.tile([68, 1024], f16, tag="b16_1")
        nc.scalar.activation(b16_1, tb1[:68], ACTF.Copy)
        bv0 = b16_0.rearrange("p (c g) -> p g c", g=4)
        bv1 = b16_1.rearrange("p (c g) -> p g c", g=4)
        for cj in range(2):
            for g in range(4):
                d1, d2 = g // 2, g % 2
                ps = pst.tile([128, 196], f16, tag="ps16")
                nc.tensor.transpose(
                    ps[:, 0:128], bv0[:, g, cj * 128:(cj + 1) * 128], ident)
                nc.tensor.transpose(
                    ps[:, 128:196], bv1[:, g, cj * 128:(cj + 1) * 128],
                    ident[:68, :68])
                dst = t_pad[:, cj, 1:29, 1:29].rearrange(
                    "p (i a) (j b) -> p a b i j", a=2, b=2)[:, d1, d2]
                nc.scalar.activation(dst, ps.rearrange("p (i j) -> p i j", j=14),
                                     ACTF.Copy)

        # ---------------- c load/pad + channel attention
        for cj in range(2):
            nc.scalar.activation(
                c_pad[:, cj, 1:29, 1:29],
                tc0[:, cj].rearrange("p (x y) -> p x y", y=28), ACTF.Copy)
        cred = smp.tile([128, 2, 2], f32, tag="cred")
        for cj in range(2):
            nc.vector.reduce_sum(cred[:, cj, 0:1], tc0[:, cj], axis=AX.X)
            nc.vector.reduce_max(cred[:, cj, 1:2], tc0[:, cj], axis=AX.X)
        v2 = smp.tile([128, 2, 2], f16, tag="v2")
        for cj in range(2):
            nc.vector.tensor_scalar_mul(v2[:, cj, 0:1], cred[:, cj, 0:1], 1.0 / 784)
            nc.vector.tensor_copy(v2[:, cj, 1:2], cred[:, cj, 1:2])
        psh = psp.tile([16, 2], f32, tag="ps")
        nc.tensor.matmul(psh, w_caw1[:, 0], v2[:, 0], start=True, stop=False)
        nc.tensor.matmul(psh, w_caw1[:, 1], v2[:, 1], start=False, stop=True)
        h16 = smp.tile([16, 2], f16, tag="h16")
        nc.scalar.activation(h16, psh, ACTF.Relu)
        cc = smp.tile([128, 2, 1], f32, tag="cc")
        for cj in range(2):
            ps2 = psp.tile([128, 1], f32, tag="ps")
            nc.tensor.matmul(ps2, w_caw2[:, cj * 128:(cj + 1) * 128], h16[:, 0:1],
                             start=True, stop=False)
            nc.tensor.matmul(ps2, w_caw2[:, cj * 128:(cj + 1) * 128], h16[:, 1:2],
                             start=False, stop=True)
            cct = smp.tile([128, 1], f32, tag="cct")
            nc.scalar.activation(cct, ps2, ACTF.Tanh, scale=0.5)
            nc.vector.tensor_scalar(cc[:, cj], cct, 0.5, 0.5,
                                    op0=ALU.mult, op1=ALU.add)

        # ---------------- spatial attention
        tadd = smp.tile([128, 28, 28], f16, tag="tadd")
        nc.vector.tensor_add(
            tadd, t_pad[:, 0, 1:29, 1:29], t_pad[:, 1, 1:29, 1:29])
        tmax = smp.tile([128, 28, 28], f16, tag="tmax")
        nc.vector.tensor_tensor(
            tmax, t_pad[:, 0, 1:29, 1:29], t_pad[:, 1, 1:29, 1:29], ALU.max)
        maxall = smp.tile([128, 28, 28], f16, tag="maxall")
        nc.gpsimd.partition_all_reduce(maxall, tmax, 128, bass_isa.ReduceOp.max)
        maxrow = maxall[0:1]
        meanrow = rwp.tile([1, 2, 392], f16, tag="meanrow")
        for h in range(2):
            psm = psr.tile([1, 14, 28], f32, tag="psrow")
            nc.tensor.matmul(
                psm, ones16_col, tadd[:, h * 14:(h + 1) * 14, :],
                start=True, stop=True)
            nc.scalar.activation(meanrow[:, h], psm.rearrange("o i j -> o (i j)"),
                                 ACTF.Copy)
        dma(sa_t[:, 0, 3:31], meanrow.rearrange("o h x -> o (h x)"))
        dma(sa_t[:, 1, 3:31], maxrow.rearrange("o x y -> o (x y)"))
        pssa = psp.tile([28, 28], f32, tag="ps")
        for ch in range(2):
            for kx in range(7):
                nc.tensor.matmul(
                    pssa, w_bands[:, ch * 7 + kx, :], sa_t[:28, ch, kx:kx + 28],
                    start=(ch == 0 and kx == 0), stop=(ch == 1 and kx == 6))
        ts28 = smp.tile([28, 28], f16, tag="ts28")
        tst = smp.tile([28, 28], f32, tag="tst")
        nc.scalar.activation(tst, pssa, ACTF.Tanh, scale=0.5)
        nc.vector.tensor_scalar(ts28, tst, 0.5, 0.5, op0=ALU.mult, op1=ALU.add)
        tsrow = rwp.tile([1, 784], f16, tag="tsrow")
        dma(tsrow, ts28.rearrange("p x -> p x"))

        # ---------------- t_x = t * cc, c_x = c * ts
        for cj in range(2):
            nc.vector.tensor_scalar_mul(tx_pad[:, cj], t_pad[:, cj], cc[:, cj])
        for h in range(2):
            psts = psp.tile([128, 14, 28], f32, tag="ps")
            nc.tensor.matmul(
                psts, ones16_row[:, 0:128],
                tsrow[:, h * 392:(h + 1) * 392].rearrange("o (i j) -> o i j", j=28),
                start=True, stop=True)
            for cj in range(2):
                r0 = 1 + h * 14
                nc.vector.tensor_tensor(
                    cx_pad[:, cj, r0:r0 + 14, 1:29],
                    c_pad[:, cj, r0:r0 + 14, 1:29], psts, ALU.mult)

        # ---------------- conv blocks
        def conv_block(lidx, in_specs, out_write):
            cb, nch = _CONV_CHUNKS[lidx]
            wt = []
            for half in range((nch + 1) // 2):
                cw = cwp.tile([128, 2, 9, CH], f16, tag="cw")
                n = min(2, nch - half * 2)
                dma(cw[:, 0:n],
                    d["d_convw"][cb + half * 2: cb + half * 2 + n].rearrange(
                        "c p t o -> p c t o"))
                wt.append(cw)
            for m in range(2):
                for h in range(2):
                    ps = psp.tile([128, 14, 28], f32, tag="ps")
                    idx = 0
                    last = nch * 9 - 1
                    for kc in range(nch):
                        tile_in, cj = in_specs[kc]
                        for dy in range(3):
                            for dx in range(3):
                                lhsT = wt[kc // 2][:, kc % 2, dy * 3 + dx,
                                                   m * 128:(m + 1) * 128]
                                rhs = tile_in[:, cj, h * 14 + dy: h * 14 + dy + 14,
                                              dx:dx + 28]
                                nc.tensor.matmul(ps, lhsT, rhs,
                                                 start=(idx == 0), stop=(idx == last))
                                idx += 1
                    out_write(m, h, ps)

        def pad_writer(dst, lidx):
            def write(m, h, ps):
                o = dst[:, m, 1 + h * 14: 15 + h * 14, 1:29]
                if h == 0:
                    nc.scalar.activation(o, ps, ACTF.Relu, bias=b_conv[:, lidx, m:m+1])
                else:
                    nc.vector.tensor_scalar(o, ps, b_conv[:, lidx, m:m+1], 0.0,
                                            op0=ALU.add, op1=ALU.max)
            return write

        conv_block(0, [(tx_pad, 0), (tx_pad, 1), (cx_pad, 0), (cx_pad, 1)],
                   pad_writer(x1_pad, 0))
        u_pad = tx_pad
        conv_block(1, [(t_pad, 0), (t_pad, 1), (x1_pad, 0), (x1_pad, 1)],
                   pad_writer(u_pad, 1))
        cx2_pad = t_pad
        conv_block(2, [(c_pad, 0), (c_pad, 1), (x1_pad, 0), (x1_pad, 1)],
                   pad_writer(cx2_pad, 2))
        x2_pad = cx_pad
        conv_block(3, [(u_pad, 0), (u_pad, 1), (cx2_pad, 0), (cx2_pad, 1)],
                   pad_writer(x2_pad, 3))
        bt = cvp.tile([128, 2, 28, 28], f16, tag="bt")

        def bt_writer(m, h, ps):
            o = bt[:, m, h * 14:(h + 1) * 14, :]
            if h == 0:
                nc.scalar.activation(o, ps, ACTF.Relu, bias=b_conv[:, 4, m:m+1])
            else:
                nc.vector.tensor_scalar(o, ps, b_conv[:, 4, m:m+1], 0.0,
                                        op0=ALU.add, op1=ALU.max)

        conv_block(4, [(x2_pad, 0), (x2_pad, 1)], bt_writer)
        btf = bt.rearrange("p m x y -> p m (x y)")

        # ---------------- cf mlp: z1 = gelu(cat @ W1 + b1); z = z1 @ W2 + b2
        z1 = tkp.tile([128, 4, 784], f16, tag="z1")
        for m in range(4):
            for h in range(2):
                ps = psp.tile([128, 392], f32, tag="ps")
                for kc in range(6):
                    rhs = (aT[:, kc, HSL[h]] if kc < 4
                           else btf[:, kc - 4, HSL[h]])
                    nc.tensor.matmul(ps, w_cf1[:, kc, m * 128:(m + 1) * 128], rhs,
                                     start=(kc == 0), stop=(kc == 5))
                nc.scalar.activation(z1[:, m, HSL[h]], ps, GELU,
                                     bias=b_cf1[:, m:m+1])
        z = tkp.tile([128, 4, 784], f32r, tag="z")
        for m in range(4):
            for h in range(2):
                ps = psp.tile([128, 392], f32, tag="ps")
                for kc in range(4):
                    nc.tensor.matmul(ps, w_cf2[:, kc, m * 128:(m + 1) * 128],
                                     z1[:, kc, HSL[h]],
                                     start=(kc == 0), stop=(kc == 3))
                nc.scalar.activation(z[:, m, HSL[h]], ps, ACTF.Identity,
                                     bias=b_cf2[:, m:m+1])

        # ---------------- layernorm helper (stats + normalize to fp16)
        def layer_norm(src, dst):
            rstd = rwp.tile([1, 2, 392], f32r, tag="rstd")
            m2 = rwp.tile([1, 2, 392], f32r, tag="m2")
            for h in range(2):
                psA = psr.tile([1, 392], f32, tag="psrow")
                psB = psr.tile([1, 392], f32, tag="psrow")
                for kc in range(4):
                    nc.tensor.matmul(psA, ones32_col, src[:, kc, HSL[h]],
                                     start=(kc == 0), stop=(kc == 3))
                for kc in range(4):
                    sqt = sqp.tile([128, 392], f32r, tag="sq")
                    nc.vector.tensor_tensor(sqt, src[:, kc, HSL[h]],
                                            src[:, kc, HSL[h]], ALU.mult)
                    nc.tensor.matmul(psB, ones32_col, sqt,
                                     start=(kc == 0), stop=(kc == 3))
                s1row = rwp.tile([1, 392], f32r, tag="s1row")
                nc.scalar.activation(s1row, psA, ACTF.Copy)
                t1 = rwp.tile([1, 392], f32r, tag="t1")
                nc.vector.tensor_tensor(t1, s1row, s1row, ALU.mult)
                nc.vector.tensor_scalar_mul(t1, t1, -1.0 / EMB)
                t2 = rwp.tile([1, 392], f32r, tag="t2")
                nc.vector.tensor_add(t2, psB, t1)
                if GELU_IDENTITY:  # CoreSim lacks Abs_reciprocal_sqrt
                    sdev = rwp.tile([1, 392], f32r, tag="t1")
                    nc.scalar.activation(sdev, t2, ACTF.Sqrt,
                                         bias=eps_row, scale=1.0 / EMB)
                    nc.vector.reciprocal(rstd[:, h], sdev)
                else:
                    nc.scalar.activation(rstd[:, h], t2, ACTF.Abs_reciprocal_sqrt,
                                         bias=eps_row, scale=1.0 / EMB)
                nc.vector.tensor_tensor(m2[:, h], s1row, rstd[:, h], ALU.mult)
                nc.vector.tensor_scalar_mul(m2[:, h], m2[:, h], 1.0 / EMB)
            for h in range(2):
                psrd = psp.tile([128, 392], f32, tag="ps")
                nc.tensor.matmul(psrd, ones32_row, rstd[:, h], start=True, stop=True)
                psm2 = psp.tile([128, 392], f32, tag="ps")
                nc.tensor.matmul(psm2, ones32_row, m2[:, h], start=True, stop=True)
                for kc in range(4):
                    tmpt = sqp.tile([128, 392], f32r, tag="sq")
                    nc.vector.tensor_tensor(tmpt, src[:, kc, HSL[h]], psrd, ALU.mult)
                    nc.vector.tensor_tensor(dst[:, kc, HSL[h]], tmpt, psm2,
                                            ALU.subtract)

        zhat = tkp.tile([128, 4, 784], f16, tag="zhat")
        layer_norm(z, zhat)

        # ---------------- kqv: k,q feature-major; v token-major
        kfm = tkp.tile([128, 4, 784], f16, tag="kfm")
        qfm = tkp.tile([128, 4, 784], f16, tag="qfm")
        for mi in range(8):
            dstT, mloc = (kfm, mi) if mi < 4 else (qfm, mi - 4)
            for h in range(2):
                ps = psp.tile([128, 392], f32, tag="ps")
                for kc in range(4):
                    nc.tensor.matmul(ps, w_kqv[:, kc, mi * 128:(mi + 1) * 128],
                                     zhat[:, kc, HSL[h]],
                                     start=(kc == 0), stop=(kc == 3))
                if h == 0:
                    nc.scalar.activation(dstT[:, mloc, HSL[h]], ps, ACTF.Identity,
                                         bias=b_kqv[:, mi:mi+1])
                else:
                    nc.vector.tensor_scalar(dstT[:, mloc, HSL[h]], ps,
                                            b_kqv[:, mi:mi+1], None, op0=ALU.add)
        vtok = tkp.tile([128, 7, 512], f16, tag="vtok")
        for ti in range(7):
            tcn = 128 if ti < 6 else 16
            tsl = slice(ti * 128, ti * 128 + tcn)
            ps = psp.tile([128, 512], f32, tag="ps")
            for kc in range(4):
                nc.tensor.matmul(ps[:tcn], zhat[:, kc, tsl], w_kqv[:, kc, 1024:1536],
                                 start=(kc == 0), stop=(kc == 3))
            nc.vector.tensor_add(vtok[:tcn, ti], ps[:tcn], w_c2vbc[:tcn])

        # ---------------- FAVOR+ feature maps
        def xd_rows(src, tag):
            xd = rwp.tile([1, 2, 392], f16, tag=tag)
            for h in range(2):
                psx = psr.tile([1, 392], f32, tag="psrow")
                for kc in range(4):
                    sqt = sqp.tile([128, 392], f16, tag="sqh")
                    nc.vector.tensor_tensor(sqt, src[:, kc, HSL[h]],
                                            src[:, kc, HSL[h]], ALU.mult)
                    nc.tensor.matmul(psx, neghalf_col, sqt,
                                     start=(kc == 0), stop=(kc == 3))
                nc.scalar.activation(xd[:, h], psx, ACTF.Copy)
            return xd

        xdk = xd_rows(kfm, "xdk")
        xdq = xd_rows(qfm, "xdq")
        kptok = tkp.tile([128, 7, 256], f16, tag="kptok")
        xdk_flat = xdk.rearrange("o h x -> o (h x)")
        for ti in range(7):
            tcn = 128 if ti < 6 else 16
            tsl = slice(ti * 128, ti * 128 + tcn)
            ps = psp.tile([128, 256], f32, tag="ps")
            for kc in range(4):
                nc.tensor.matmul(ps[:tcn], kfm[:, kc, tsl], w_wT[:, kc],
                                 start=(kc == 0), stop=False)
            nc.tensor.matmul(ps[:tcn], xdk_flat[:, tsl], ones16_row,
                             start=False, stop=True)
            nc.scalar.activation(kptok[:tcn, ti], ps[:tcn], ACTF.Exp,
                                 bias=negln16_col[:tcn])
        qp = tkp.tile([128, 2, 784], f16, tag="qp")
        for mi in range(2):
            for h in range(2):
                ps = psp.tile([128, 392], f32, tag="ps")
                for kc in range(4):
                    nc.tensor.matmul(ps, w_wT[:, kc, mi * 128:(mi + 1) * 128],
                                     qfm[:, kc, HSL[h]],
                                     start=(kc == 0), stop=False)
                nc.tensor.matmul(ps, ones16_row[:, 0:128], xdq[:, h],
                                 start=False, stop=True)
                nc.scalar.activation(qp[:, mi, HSL[h]], ps, ACTF.Exp,
                                     bias=negln16_col)

        # ---------------- ksum, D, kptv, y
        kscol = smp.tile([128, 2, 1], f16, tag="kscol")
        for mc in range(2):
            psk = psp.tile([128, 1], f32, tag="ps")
            for ti in range(7):
                tcn = 128 if ti < 6 else 16
                nc.tensor.matmul(psk, kptok[:tcn, ti, mc * 128:(mc + 1) * 128],
                                 ones16_col[:tcn], start=(ti == 0), stop=(ti == 6))
            nc.vector.tensor_copy(kscol[:, mc], psk)
        invd = rwp.tile([1, 2, 392], f32r, tag="invd")
        for h in range(2):
            psD = psr.tile([1, 392], f32, tag="psrow")
            for mc in range(2):
                nc.tensor.matmul(psD, kscol[:, mc], qp[:, mc, HSL[h]],
                                 start=(mc == 0), stop=(mc == 1))
            if GELU_IDENTITY:
                dreg = rwp.tile([1, 392], f32r, tag="t1")
                nc.vector.tensor_scalar_add(dreg, psD, 1e-8)
                nc.vector.reciprocal(invd[:, h], dreg)
            else:
                dr = rwp.tile([1, 392], f32r, tag="t1")
                nc.scalar.activation(dr, psD, ACTF.Abs_reciprocal_sqrt,
                                     bias=eps8_row)
                nc.vector.tensor_tensor(invd[:, h], dr, dr, ALU.mult)
        kptvT = tkp.tile([128, 2, 512], f16, tag="kptvT")
        for mc in range(2):
            ps = psp.tile([128, 512], f32, tag="ps")
            for ti in range(7):
                tcn = 128 if ti < 6 else 16
                nc.tensor.matmul(ps, kptok[:tcn, ti, mc * 128:(mc + 1) * 128],
                                 vtok[:tcn, ti], start=(ti == 0), stop=(ti == 6))
            nc.scalar.activation(kptvT[:, mc], ps, ACTF.Copy)
        y = tkp.tile([128, 4, 784], f16, tag="kfm")
        for h in range(2):
            psb = psp.tile([128, 392], f32, tag="ps")
            nc.tensor.matmul(psb, ones32_row, invd[:, h], start=True, stop=True)
            invb = sqp.tile([128, 392], f32r, tag="sq")
            nc.scalar.activation(invb, psb, ACTF.Copy)
            for vc in range(4):
                psy = psp.tile([128, 392], f32, tag="ps")
                for mc in range(2):
                    nc.tensor.matmul(psy, kptvT[:, mc, vc * 128:(vc + 1) * 128],
                                     qp[:, mc, HSL[h]],
                                     start=(mc == 0), stop=(mc == 1))
                nc.vector.tensor_tensor(y[:, vc, HSL[h]], psy, invb, ALU.mult)

        # ---------------- proj + residual into z
        for m in range(4):
            for h in range(2):
                ps = psp.tile([128, 392], f32, tag="ps")
                for kc in range(4):
                    nc.tensor.matmul(ps, w_proj[:, kc, m * 128:(m + 1) * 128],
                                     y[:, kc, HSL[h]],
                                     start=(kc == 0), stop=(kc == 3))
                tmp = sqp.tile([128, 392], f32r, tag="sq")
                nc.scalar.activation(tmp, ps, ACTF.Identity, bias=b_proj[:, m:m+1])
                nc.vector.tensor_add(z[:, m, HSL[h]], z[:, m, HSL[h]], tmp)

        # ---------------- LN2 + mlp
        zhat2 = tkp.tile([128, 4, 784], f16, tag="zhat")
        layer_norm(z, zhat2)
        g1t = tkp.tile([128, 4, 784], f16, tag="z1")
        for m in range(4):
            for h in range(2):
                ps = psp.tile([128, 392], f32, tag="ps")
                for kc in range(4):
                    nc.tensor.matmul(ps, w_mlp1[:, kc, m * 128:(m + 1) * 128],
                                     zhat2[:, kc, HSL[h]],
                                     start=(kc == 0), stop=(kc == 3))
                nc.scalar.activation(g1t[:, m, HSL[h]], ps, GELU,
                                     bias=b_mlp1[:, m:m+1])
        # mlp2 feature-major + residual; output feature-major (host transposes)
        z3 = tkp.tile([128, 4, 784], f32r, tag="z3")
        for m in range(4):
            for h in range(2):
                ps = psp.tile([128, 392], f32, tag="ps")
                for kc in range(4):
                    nc.tensor.matmul(ps, w_mlp2[:, kc, m * 128:(m + 1) * 128],
                                     g1t[:, kc, HSL[h]],
                                     start=(kc == 0), stop=(kc == 3))
                tmp = sqp.tile([128, 392], f32r, tag="sq")
                nc.scalar.activation(tmp, ps, ACTF.Identity, bias=b_mlp2[:, m:m+1])
                nc.vector.tensor_add(z3[:, m, HSL[h]], z[:, m, HSL[h]], tmp)
        dma(d["d_out"][it].rearrange("(c p) t -> p c t", p=128), z3.bitcast(f32))


# ---------------------------------------------------------------- entry
_BUILD_CACHE = {}


def _get_nc(n_items):
    if n_items not in _BUILD_CACHE:
        _BUILD_CACHE[n_items] = _build_bass(n_items)
    return _BUILD_CACHE[n_items]


def kernel(a, b, c, params, w):
    from concourse.bass_utils import run_bass_kernel_spmd

    arrs = _prep_arrays(a, b, c, params, w)
    nc = _get_nc(ITEMS)

    shared = {k: v for k, v in arrs.items() if k not in ("a_t", "b_in", "c_in")}
    in_maps = []
    for core in range(N_CORES):
        s = slice(core * ITEMS, (core + 1) * ITEMS)
        m = dict(shared)
        m["a_t"] = arrs["a_t"][s]
        m["b_in"] = arrs["b_in"][s]
        m["c_in"] = arrs["c_in"][s]
        in_maps.append(m)

    res = run_bass_kernel_spmd(nc, in_maps, core_ids=list(range(N_CORES)))
    global LAST_RESULT
    LAST_RESULT = res
    out = np.concatenate([r["out"] for r in res.results], axis=0)
    return np.ascontiguousarray(out.transpose(0, 2, 1), dtype=np.float32)


LAST_RESULT = None
